# revision 49
# baseline (speedup 1.0000x reference)
"""GNN message-passing kernel for TRN2, 8-core SPMD (self-contained).

v4 design (on top of v3), ~1.3 ms vs the 2.17 ms v3 baseline:
- Node rows sharded 8 ways (NS=N/8), edge rows too (ES=E/8).
- Mixed-precision gathers: node-adjacency gathers are bf16 at hops 1-2 and
  fp8-e4m3 at hop 0 (the hop-0 table is a host-quantized input, so no AG
  cost); edge-embedding gathers (node phase) and node-dep gathers (edge
  phase) are fp8 everywhere, halving their DMA bytes. CPU-sim rel err of
  this split 1.25e-2, HW 1.27e-2 (< 2e-2 gate).
- fp8 selector matmuls run pairwise in DoubleRow perf mode (2 fp8 weights
  per PE cell): ~2 chunks per 239 ns instead of 2x370 ns.
- Gather counts are compile-time per-piece maxima across cores; shorter
  cores pad with fake idx-0/dest=-1 entries, and the index tail beyond the
  shared count is -1 (SWDGE skips negative tails entirely). Gather-ring
  SBUF is memset once at startup so skipped tails can never feed NaNs into
  the zero-selector matmuls.
- AllGather restructure: small fp8 node tables (needed by the very next
  edge phase) gather right behind the producing node blocks; bf16 node
  tables (needed only by the NEXT hop's adjacency gathers) gather during
  the edge phase, off the critical path. The edge table is split 5/8 : 3/8
  into lo / hi part-tables (separate Shared tensors, host-remapped
  indices): the lo AllGather hides under the remaining edge blocks and the
  smaller hi AllGather shortens the exposed hop-boundary tail; the next
  hop's adjacency + lo-part gathers are prefetched before the hi AllGather
  so they run during it.
- Segment-mean via selector matmuls on the Tensor engine (is_equal-built
  0/1 selectors on the DVE), 1/cnt folded into the PSUM->SBUF activation
  copy. Linear layers bf16, bias via rank-1 matmul, ReLU fused in the
  PSUM->SBUF copy.
"""
import sys

sys.path.insert(0, '/opt/trn_rl_repo')

import numpy as np
import concourse.bass as bass
import concourse.mybir as mybir
from concourse import tile
from concourse.bacc import Bacc
from concourse.masks import make_identity

F32 = mybir.dt.float32
I32 = mybir.dt.int32
BF16 = mybir.dt.bfloat16
FP8 = mybir.dt.float8e4
I16 = mybir.dt.int16
P = 128

CMAX = 8  # max 128-row chunks per dma_gather call (ring capacity)


class Cfg:
    def __init__(self, N=8192, E=32768, D=512, DEG=16, DEP=8, K=3, CORES=8):
        self.N, self.E, self.D = N, E, D
        self.DEG, self.DEP, self.K, self.CORES = DEG, DEP, K, CORES
        self.NS = N // CORES
        self.ES = E // CORES
        self.NB = self.NS // P
        self.EB = self.ES // P
        self.DC = D // P
        self.KCN = (2 * D) // P
        self.KCE = (3 * D) // P
        assert self.NS % P == 0 and self.ES % P == 0 and D % P == 0
        assert 2 * N <= 32768 and E <= 32768  # int16 dma_gather indices


def _ceil128(x):
    return -(-x // 128)


def _pieces(n):
    out = []
    off = 0
    while off < n:
        out.append(min(CMAX, n - off))
        off += CMAX
    return out


class Plan:
    """Host-derived compile-time structure (chunk counts, column offsets),
    maxed across cores so one SPMD program fits all cores."""

    def __init__(self, cfg, inputs):
        NS, ES, NB, EB = cfg.NS, cfg.ES, cfg.NB, cfg.EB
        C = cfg.CORES
        adj = {0: np.asarray(inputs["fw_adj"], np.int64),
               1: np.asarray(inputs["bw_adj"], np.int64)}
        eid = {0: np.asarray(inputs["fw_edgeid"], np.int64),
               1: np.asarray(inputs["bw_edgeid"], np.int64)}
        dep = {0: np.asarray(inputs["fw_edgedep"], np.int64),
               1: np.asarray(inputs["bw_edgedep"], np.int64)}

        ES_LO = (ES * 5) // 8

        def e_half(v):
            # edge id -> which half-table it lives in (-1 for padding)
            return np.where(v < 0, -1, ((v % ES) >= ES_LO).astype(np.int64))

        self.Ka = np.zeros((2, NB), np.int64)
        self.Ke = np.zeros((2, 2, NB), np.int64)  # [half, d, b]
        self.Kf = np.zeros(EB, np.int64)
        self.Kb = np.zeros(EB, np.int64)
        for d in (0, 1):
            for b in range(NB):
                for c in range(C):
                    r0 = c * NS + b * P
                    self.Ka[d, b] = max(self.Ka[d, b],
                                        _ceil128((adj[d][r0:r0 + P] >= 0).sum()))
                    eh = e_half(eid[d][r0:r0 + P])
                    for h in (0, 1):
                        self.Ke[h, d, b] = max(self.Ke[h, d, b],
                                               _ceil128((eh == h).sum()))
        for b in range(EB):
            for c in range(C):
                r0 = c * ES + b * P
                self.Kf[b] = max(self.Kf[b], _ceil128((dep[0][r0:r0 + P] >= 0).sum()))
                self.Kb[b] = max(self.Kb[b], _ceil128((dep[1][r0:r0 + P] >= 0).sum()))

        self.n_off = np.zeros((2, NB), np.int64)
        off = 0
        for d in (0, 1):
            for b in range(NB):
                self.n_off[d, b] = off
                off += self.Ka[d, b] + self.Ke[0, d, b] + self.Ke[1, d, b]
        self.n_chunks = off
        self.e_off = np.zeros(EB, np.int64)
        off = 0
        for b in range(EB):
            self.e_off[b] = off
            off += self.Kf[b] + self.Kb[b]
        self.e_chunks = off

        # Per-piece transfer counts: max over cores of the piece's valid
        # count (compile-time constants; shorter cores pad with fake idx-0 /
        # dest=-1 entries up to the max, -1 skip-tail beyond). Keyed by
        # (kind, d_or_none, b, piece_idx) in issue order per block.
        def counts(vals_by_core, kch):
            per_core = [int((v >= 0).sum()) for v in vals_by_core]
            cnts = []
            off = 0
            for nch in _pieces(kch):
                c = max(min(max(vc - off * 128, 0), nch * 128)
                        for vc in per_core)
                cnts.append(max(c, 1))
                off += nch
            return cnts

        self.cnt_a = {}
        self.cnt_e = {}
        for d in (0, 1):
            for b in range(NB):
                rows = [adj[d][c * NS + b * P: c * NS + (b + 1) * P]
                        for c in range(C)]
                self.cnt_a[d, b] = counts(rows, int(self.Ka[d, b]))
                for h in (0, 1):
                    rows = [np.where(
                        e_half(eid[d][c * NS + b * P: c * NS + (b + 1) * P])
                        == h, 0, -1) for c in range(C)]
                    self.cnt_e[h, d, b] = counts(rows, int(self.Ke[h, d, b]))
        self.cnt_f = {}
        self.cnt_b = {}
        for b in range(EB):
            rows = [dep[0][c * ES + b * P: c * ES + (b + 1) * P]
                    for c in range(C)]
            self.cnt_f[b] = counts(rows, int(self.Kf[b]))
            rows = [dep[1][c * ES + b * P: c * ES + (b + 1) * P]
                    for c in range(C)]
            self.cnt_b[b] = counts(rows, int(self.Kb[b]))

        self.sig = (tuple(self.Ka.ravel()), tuple(self.Ke.ravel()),
                    tuple(self.Kf), tuple(self.Kb),
                    tuple(tuple(v) for v in self.cnt_a.values()),
                    tuple(tuple(v) for v in self.cnt_e.values()),
                    tuple(tuple(v) for v in self.cnt_f.values()),
                    tuple(tuple(v) for v in self.cnt_b.values()))


def build(cfg: Cfg, plan: Plan, pf=4):
    N, E, D = cfg.N, cfg.E, cfg.D
    K, CORES = cfg.K, cfg.CORES
    NS, ES, NB, EB = cfg.NS, cfg.ES, cfg.NB, cfg.EB
    DC, KCN, KCE = cfg.DC, cfg.KCN, cfg.KCE
    Ka, Ke, Kf, Kb = plan.Ka, plan.Ke, plan.Kf, plan.Kb
    KA_MAX = min(int(Ka.max()), CMAX)
    KE_MAX = min(int(Ke.max()), CMAX)
    KD_MAX = min(int(max(Kf.max(), Kb.max())), CMAX)
    SELA_MAX = int(Ka.max())
    SELE_MAX = int((Ke[0] + Ke[1]).max())
    SELD_MAX = int((Kf + Kb).max())
    ES_LO = (cfg.ES * 5) // 8
    ES_HI = cfg.ES - ES_LO
    E_LO = CORES * ES_LO
    E_HI = CORES * ES_HI

    nc = Bacc("TRN2", target_bir_lowering=False, debug=False, num_devices=CORES,
              num_swdge_queues=4)

    # ---- external inputs ----
    fw_tab0 = nc.dram_tensor("fw_tab0", [N, D], FP8, kind="ExternalInput")
    bw_tab0 = nc.dram_tensor("bw_tab0", [N, D], FP8, kind="ExternalInput")
    e_tab0_lo = nc.dram_tensor("e_tab0_lo", [E_LO, D], FP8, kind="ExternalInput")
    e_tab0_hi = nc.dram_tensor("e_tab0_hi", [E_HI, D], FP8, kind="ExternalInput")
    fw_own0 = nc.dram_tensor("fw_own0", [NS, D], BF16, kind="ExternalInput")
    bw_own0 = nc.dram_tensor("bw_own0", [NS, D], BF16, kind="ExternalInput")
    e_own0 = nc.dram_tensor("e_own0", [ES, D], BF16, kind="ExternalInput")
    idx_n = nc.dram_tensor("idx_n", [P, plan.n_chunks * 8], I16, kind="ExternalInput")
    idx_e = nc.dram_tensor("idx_e", [P, plan.e_chunks * 8], I16, kind="ExternalInput")
    dest_n = nc.dram_tensor("dest_n", [P, plan.n_chunks], BF16, kind="ExternalInput")
    dest_e = nc.dram_tensor("dest_e", [P, plan.e_chunks], BF16, kind="ExternalInput")
    rcn_x = nc.dram_tensor("rcn", [P, 2 * NB], F32, kind="ExternalInput")
    rcef_x = nc.dram_tensor("rcef", [P, EB], F32, kind="ExternalInput")
    rceb_x = nc.dram_tensor("rceb", [P, EB], F32, kind="ExternalInput")
    wfc_x = nc.dram_tensor("wfc", [P, KCN * D], BF16, kind="ExternalInput")
    wbc_x = nc.dram_tensor("wbc", [P, KCN * D], BF16, kind="ExternalInput")
    wed_x = nc.dram_tensor("wed", [P, KCE * D], BF16, kind="ExternalInput")
    bfc_x = nc.dram_tensor("bfc", [1, D], BF16, kind="ExternalInput")
    bbc_x = nc.dram_tensor("bbc", [1, D], BF16, kind="ExternalInput")
    bed_x = nc.dram_tensor("bed", [1, D], BF16, kind="ExternalInput")
    fw_out = nc.dram_tensor("fw_out", [NS, D], F32, kind="ExternalOutput")
    bw_out = nc.dram_tensor("bw_out", [NS, D], F32, kind="ExternalOutput")

    rg = [list(range(CORES))]
    RELU = mybir.ActivationFunctionType.Relu
    COPY = mybir.ActivationFunctionType.Copy
    EQ = mybir.AluOpType.is_equal

    with tile.TileContext(nc) as tc:
        with (
            tc.tile_pool(name="const", bufs=1) as cp,
            tc.tile_pool(name="gp", bufs=2) as gp,
            tc.tile_pool(name="slp", bufs=3) as slp,
            tc.tile_pool(name="sp", bufs=3) as sp,
            tc.tile_pool(name="xp", bufs=2) as xp,
            tc.tile_pool(name="op", bufs=3) as op,
            tc.tile_pool(name="pm", bufs=2, space="PSUM") as pmp,
            tc.tile_pool(name="pt", bufs=1, space="PSUM") as ptp,
            tc.tile_pool(name="po", bufs=2, space="PSUM") as pop,
            tc.tile_pool(name="dram", bufs=1, space="DRAM") as dp,
        ):
            # ---- constants ----
            ident = cp.tile([P, P], BF16)
            make_identity(nc, ident[:])
            ones1 = cp.tile([1, P], BF16)
            nc.gpsimd.memset(ones1[:], 1.0)
            iota_i = cp.tile([P, P], I32)
            nc.gpsimd.iota(iota_i[:], pattern=[[1, P]], base=0,
                           channel_multiplier=0)
            iota_b = cp.tile([P, P], BF16)
            nc.vector.tensor_copy(out=iota_b[:], in_=iota_i[:])

            def load_flat(name, src, shape, dt):
                t = cp.tile(shape, dt, name=name)
                nc.sync.dma_start(out=t[:], in_=src[:])
                return t

            ixn_t = load_flat("ixn_t", idx_n, [P, plan.n_chunks * 8], I16)
            dn_t = load_flat("dn_t", dest_n, [P, plan.n_chunks], BF16)
            rcn_t = load_flat("rcn_t", rcn_x, [P, 2 * NB], F32)
            ixe_t = load_flat("ixe_t", idx_e, [P, plan.e_chunks * 8], I16)
            de_t = load_flat("de_t", dest_e, [P, plan.e_chunks], BF16)
            wfc_t = load_flat("wfc_t", wfc_x, [P, KCN * D], BF16)
            wbc_t = load_flat("wbc_t", wbc_x, [P, KCN * D], BF16)
            wed_t = load_flat("wed_t", wed_x, [P, KCE * D], BF16)
            bfc_t = load_flat("bfc_t", bfc_x, [1, D], BF16)
            bbc_t = load_flat("bbc_t", bbc_x, [1, D], BF16)
            bed_t = load_flat("bed_t", bed_x, [1, D], BF16)
            rcef_t = load_flat("rcef_t", rcef_x, [P, EB], F32)
            rceb_t = load_flat("rceb_t", rceb_x, [P, EB], F32)

            # ---- DRAM tables / staging ----
            fw_tabs = [fw_tab0] + [dp.tile([N, D], BF16, addr_space="Shared",
                                           name=f"fw_tab{k}") for k in (1, 2)]
            bw_tabs = [bw_tab0] + [dp.tile([N, D], BF16, addr_space="Shared",
                                           name=f"bw_tab{k}") for k in (1, 2)]
            n_tabs = [(fw_tabs[k], bw_tabs[k]) for k in range(K)]
            # per-direction fp8 node tables (edge-phase dep gathers): the
            # bw AllGather triggers right after the bw node blocks and hides
            # under the fw node phase, so edge gb gathers start immediately.
            fw_tabq = [None] + [dp.tile([N, D], FP8, addr_space="Shared",
                                        name=f"fw_tq{k}") for k in (1, 2)]
            bw_tabq = [None] + [dp.tile([N, D], FP8, addr_space="Shared",
                                        name=f"bw_tq{k}") for k in (1, 2)]
            e_tabs = [(e_tab0_lo, e_tab0_hi)] + [
                (dp.tile([E_LO, D], FP8, addr_space="Shared", name=f"e_tl{k}"),
                 dp.tile([E_HI, D], FP8, addr_space="Shared", name=f"e_th{k}"))
                for k in (1, 2)]
            fw_st = [fw_own0, dp.tile([NS, D], BF16, name="fw_shA"),
                     dp.tile([NS, D], BF16, name="fw_shB")]
            bw_st = [bw_own0, dp.tile([NS, D], BF16, name="bw_shA"),
                     dp.tile([NS, D], BF16, name="bw_shB")]
            fw_stq = [None, dp.tile([NS, D], FP8, name="fw_qA"),
                      dp.tile([NS, D], FP8, name="fw_qB")]
            bw_stq = [None, dp.tile([NS, D], FP8, name="bw_qA"),
                      dp.tile([NS, D], FP8, name="bw_qB")]
            e_st = [e_own0, dp.tile([ES, D], BF16, name="e_shA"),
                    dp.tile([ES, D], BF16, name="e_shB")]
            e_stq = [None, dp.tile([ES, D], FP8, name="e_qA"),
                     dp.tile([ES, D], FP8, name="e_qB")]

            qctr = [0]

            def gather(tab_ap, idx_tile, chunk_off, nchunks, tag, maxch, dt,
                       cnts, bufs=None, into=None, into_col=0,
                       full_cnt=False):
                """ceil(nchunks/CMAX) dma_gather calls -> [(tile, col, nch)].
                cnts[i] = compile-time transfer count (max across cores).
                into/into_col: write into an existing tile at a chunk col."""
                out = []
                off = 0
                pi = 0
                while off < nchunks:
                    nch = min(CMAX, nchunks - off)
                    if into is None:
                        g = gp.tile([P, min(maxch, CMAX) * D], dt,
                                    name=f"g_{tag}", tag=tag, bufs=bufs)
                        col = 0
                    else:
                        g = into
                        col = into_col + off
                    qctr[0] = (qctr[0] + 1) % 4
                    nc.gpsimd.dma_gather(
                        out_ap=g[:, col * D:(col + nch) * D]
                            .rearrange("p (t e) -> p t e", e=D),
                        in_ap=tab_ap,
                        idxs_ap=idx_tile[:, (chunk_off + off) * 8:
                                         (chunk_off + off + nch) * 8],
                        num_idxs=nch * P,
                        num_idxs_reg=nch * P if full_cnt else int(cnts[pi]),
                        elem_size=D,
                        queue_num=qctr[0],
                    )
                    out.append((g, col, nch))
                    off += nch
                    pi += 1
                return out

            def allgather(src_ap, dst_ap):
                nc.gpsimd.collective_compute(
                    "AllGather", mybir.AluOpType.bypass, replica_groups=rg,
                    ins=[src_ap], outs=[dst_ap],
                )

            def ag_rows(st, tab, rows_total, r0, r1):
                """AllGather staging rows [r0:r1) into the strided full-table
                view [C, rows_total, D][:, r0:r1, :]."""
                dst = tab[:].rearrange("(c r) d -> c r d", r=rows_total)
                allgather(st[r0:r1, :], dst[:, r0:r1, :])

            def build_sel(dtile, co, nch, dt, tag, smax):
                """[128, nch*128] selector: sel[r, c*128+p] =
                (dest[r, co+c] == p)."""
                st = slp.tile([P, smax * P], dt, name=f"sel_{tag}", tag=tag)
                io_b = iota_b[:].rearrange("p (o f) -> p o f", o=1) \
                                .broadcast_to([P, nch, P])
                db = dtile[:, co:co + nch].rearrange("p (c o) -> p c o", o=1) \
                                          .broadcast_to([P, nch, P])
                nc.vector.tensor_tensor(
                    out=st[:, :nch * P].rearrange("p (c f) -> p c f", f=P),
                    in0=io_b, in1=db, op=EQ)
                return st

            def flat_chunks(glist):
                return [(g, col + c) for g, col, n in glist for c in range(n)]

            DR = mybir.MatmulPerfMode.DoubleRow

            def sel_matmul(ps, sel_t, c0, chunks, first, last):
                # pair adjacent fp8 chunks from the same gather tile into
                # DoubleRow matmuls (2 fp8 weights per PE cell)
                groups = []
                i = 0
                while i < len(chunks):
                    g, c = chunks[i]
                    if (sel_t.dtype == FP8 and i + 1 < len(chunks)
                            and chunks[i + 1][0] is g
                            and chunks[i + 1][1] == c + 1):
                        groups.append((g, c, i, True))
                        i += 2
                    else:
                        groups.append((g, c, i, False))
                        i += 1
                for gi, (g, c, i, dbl) in enumerate(groups):
                    st = first and gi == 0
                    sp_ = last and gi == len(groups) - 1
                    if dbl:
                        nc.tensor.matmul(
                            out=ps,
                            lhsT=sel_t[:, (c0 + i) * P:(c0 + i + 2) * P]
                                .rearrange("p (k m) -> p k m", k=2),
                            rhs=g[:, c * D:(c + 2) * D]
                                .rearrange("p (k d) -> p k d", k=2),
                            start=st, stop=sp_, perf_mode=DR,
                        )
                    else:
                        nc.tensor.matmul(
                            out=ps,
                            lhsT=sel_t[:, (c0 + i) * P:(c0 + i + 1) * P],
                            rhs=g[:, c * D:(c + 1) * D],
                            start=st, stop=sp_,
                        )

            def transpose_into(pt, cbase, src_sb, nch):
                for c in range(nch):
                    nc.tensor.transpose(
                        out=pt[:, (cbase + c) * P:(cbase + c + 1) * P],
                        in_=src_sb[:, c * P:(c + 1) * P],
                        identity=ident[:],
                    )

            def linear(xT, kc, w_t, b_row):
                ps = pop.tile([P, D], F32, name="ps_o", tag="ps_o")
                for kk in range(kc):
                    nc.tensor.matmul(
                        out=ps[:], lhsT=xT[:, kk * P:(kk + 1) * P],
                        rhs=w_t[:, kk * D:(kk + 1) * D],
                        start=(kk == 0), stop=False,
                    )
                nc.tensor.matmul(out=ps[:], lhsT=ones1[:], rhs=b_row[:],
                                 start=False, stop=True)
                return ps

            GA_BUFS = 2 * (pf + 1)

            # Zero every gather-ring buffer once: skipped -1 tails leave
            # stale SBUF that the selector matmuls read (zero-selector), and
            # uninitialized bits could decode as NaN (0 * NaN = NaN).
            for i in range(GA_BUFS):
                t = gp.tile([P, min(KA_MAX, CMAX) * D], BF16, name="z_ga",
                            tag="ga", bufs=GA_BUFS)
                eng = nc.vector if i % 2 == 0 else nc.gpsimd
                eng.memset(t[:], 0.0)
            for i in range(4):
                t = gp.tile([P, min(KE_MAX, CMAX) * D], FP8, name="z_ge",
                            tag="ge", bufs=4)
                eng = nc.gpsimd if i % 2 == 0 else nc.vector
                eng.memset(t[:], 0.0)

            def node_adj_gather(k, d, b):
                tab = n_tabs[k][d]
                dt = FP8 if k == 0 else BF16
                return gather(tab[:], ixn_t, int(plan.n_off[d, b]),
                              int(Ka[d, b]), "ga", KA_MAX, dt,
                              plan.cnt_a[d, b], bufs=GA_BUFS)

            def node_ge_lo(k, d, b):
                ke0 = int(Ke[0, d, b])
                co = int(plan.n_off[d, b]) + int(Ka[d, b])
                lo = gather(e_tabs[k][0][:], ixn_t, co, ke0, "ge", KE_MAX,
                            FP8, plan.cnt_e[0, d, b], bufs=4)
                return True, lo

            def node_ge_hi(k, d, b, gt):
                ke0, ke1 = int(Ke[0, d, b]), int(Ke[1, d, b])
                co = int(plan.n_off[d, b]) + int(Ka[d, b])
                return gather(e_tabs[k][1][:], ixn_t, co + ke0, ke1, "ge",
                              KE_MAX, FP8, plan.cnt_e[1, d, b], bufs=4)

            def node_block(k, d, b, ga, ge):
                last = (k == K - 1)
                ka = int(Ka[d, b])
                ke0, ke1 = int(Ke[0, d, b]), int(Ke[1, d, b])
                ke = ke0 + ke1
                co = int(plan.n_off[d, b])
                adt = FP8 if k == 0 else BF16
                sel_a = build_sel(dn_t, co, ka, adt, "sela", SELA_MAX)
                sel_e = build_sel(dn_t, co + ka, ke, FP8, "sele", SELE_MAX)
                own = sp.tile([P, D], BF16, name="own", tag="own")
                st = fw_st[k] if d == 0 else bw_st[k]
                nc.sync.dma_start(out=own[:], in_=st[b * P:(b + 1) * P, :])

                ps_m = pmp.tile([P, D], F32, name="ps_m", tag="ps_f")
                sel_matmul(ps_m[:], sel_a, 0, flat_chunks(ga),
                           True, ke == 0)
                sel_matmul(ps_m[:], sel_e, 0, flat_chunks(ge),
                           ka == 0, True)
                m_sb = sp.tile([P, D], BF16, name="m_sb", tag="m")
                nc.scalar.activation(out=m_sb[:], in_=ps_m[:], func=COPY,
                                     scale=rcn_t[:, d * NB + b:d * NB + b + 1])

                pt = ptp.tile([P, KCN * P], BF16, name="pt", tag="pt")
                transpose_into(pt, 0, own[:], DC)
                transpose_into(pt, DC, m_sb[:], DC)
                xT = xp.tile([P, KCN * P], BF16, name="xT", tag="xT")
                nc.vector.tensor_copy(out=xT[:], in_=pt[:])

                w_t = wfc_t if d == 0 else wbc_t
                b_row = bfc_t if d == 0 else bbc_t
                ps_o = linear(xT, KCN, w_t, b_row)
                if not last:
                    ob = op.tile([P, D], BF16, name="ob", tag="ob")
                    nc.scalar.activation(out=ob[:], in_=ps_o[:], func=RELU)
                    obq = op.tile([P, D], FP8, name="obq", tag="obq")
                    nc.scalar.activation(out=obq[:], in_=ps_o[:], func=RELU)
                    dst = fw_st[k + 1] if d == 0 else bw_st[k + 1]
                    dstq = fw_stq[k + 1] if d == 0 else bw_stq[k + 1]
                    nc.sync.dma_start(out=dst[b * P:(b + 1) * P, :], in_=ob[:])
                    nc.sync.dma_start(out=dstq[b * P:(b + 1) * P, :],
                                      in_=obq[:])
                else:
                    of = op.tile([P, D], F32, name="of", tag="of")
                    nc.scalar.activation(out=of[:], in_=ps_o[:], func=COPY)
                    dst = fw_out if d == 0 else bw_out
                    nc.sync.dma_start(out=dst[b * P:(b + 1) * P, :], in_=of[:])

            def edge_gb_gather(k, b):
                kf, kb = int(Kf[b]), int(Kb[b])
                co = int(plan.e_off[b])
                return gather(bw_tabq[k + 1][:], ixe_t, co + kf, kb, "gd",
                              KD_MAX, FP8, plan.cnt_b[b], bufs=8)

            def edge_gf_gather(k, b):
                kf = int(Kf[b])
                co = int(plan.e_off[b])
                return gather(fw_tabq[k + 1][:], ixe_t, co, kf, "gd",
                              KD_MAX, FP8, plan.cnt_f[b], bufs=8)

            def edge_block(k, b, gb, gf):
                kf, kb = int(Kf[b]), int(Kb[b])
                co = int(plan.e_off[b])
                sel_t = build_sel(de_t, co, kf + kb, FP8, "seld", SELD_MAX)
                own = sp.tile([P, D], BF16, name="own_e", tag="own")
                nc.sync.dma_start(out=own[:],
                                  in_=e_st[k][b * P:(b + 1) * P, :])

                # bw half first, fw half second (independent PSUM tiles so
                # each half retires on its own).
                ps_b = pmp.tile([P, D], F32, name="ps_be", tag="ps_b")
                sel_matmul(ps_b[:], sel_t, kf, flat_chunks(gb),
                           True, True)
                mb = sp.tile([P, D], BF16, name="mb", tag="m2")
                nc.scalar.activation(out=mb[:], in_=ps_b[:], func=COPY,
                                     scale=rceb_t[:, b:b + 1])

                ps_f = pmp.tile([P, D], F32, name="ps_fe", tag="ps_f")
                sel_matmul(ps_f[:], sel_t, 0, flat_chunks(gf),
                           True, True)
                mf = sp.tile([P, D], BF16, name="mf", tag="m")
                nc.scalar.activation(out=mf[:], in_=ps_f[:], func=COPY,
                                     scale=rcef_t[:, b:b + 1])

                pt = ptp.tile([P, KCE * P], BF16, name="pt_e", tag="pt")
                transpose_into(pt, 0, own[:], DC)
                transpose_into(pt, DC, mf[:], DC)
                transpose_into(pt, 2 * DC, mb[:], DC)
                xT = xp.tile([P, KCE * P], BF16, name="xT_e", tag="xT")
                nc.vector.tensor_copy(out=xT[:], in_=pt[:])

                ps_o = linear(xT, KCE, wed_t, bed_t)
                eb = op.tile([P, D], BF16, name="eb", tag="ob")
                nc.scalar.activation(out=eb[:], in_=ps_o[:], func=RELU)
                ebq = op.tile([P, D], FP8, name="ebq", tag="obq")
                nc.scalar.activation(out=ebq[:], in_=ps_o[:], func=RELU)
                nc.sync.dma_start(out=e_st[k + 1][b * P:(b + 1) * P, :],
                                  in_=eb[:])
                nc.sync.dma_start(out=e_stq[k + 1][b * P:(b + 1) * P, :],
                                  in_=ebq[:])

            # ---------------- program ----------------
            def prefetch_unit(k, d, b, with_lo):
                ent = {"ga": node_adj_gather(k, d, b), "gt": None, "lo": None}
                if with_lo:
                    ent["gt"], ent["lo"] = node_ge_lo(k, d, b)
                return ent

            units = [(d, b) for d in (1, 0) for b in range(NB)]
            pend = [prefetch_unit(0, *units[j], with_lo=(j < 2))
                    for j in range(pf)]
            EPF = 6
            FPF = 3
            for k in range(K):
                epend = None
                for ui, (d, b) in enumerate(units):
                    if k == 0 and ui == 0:
                        # edge-gather ring is first touched in the edge
                        # phase: zero it during the node phase
                        for i in range(8):
                            t = gp.tile([P, min(KD_MAX, CMAX) * D], FP8,
                                        name="z_gd", tag="gd", bufs=8)
                            nc.vector.memset(t[:], 0.0)
                    ent = pend[ui]
                    if ui + pf < len(units):
                        pend.append(
                            prefetch_unit(k, *units[ui + pf], with_lo=False))
                    if ent["gt"] is None:
                        ent["gt"], ent["lo"] = node_ge_lo(k, d, b)
                    ge = ent["lo"] + node_ge_hi(k, d, b, ent["gt"])
                    node_block(k, d, b, ent["ga"], ge)
                    if k < K - 1 and b == NB - 1:
                        # per-direction fp8 AllGather right behind its last
                        # producing block (bw first, so its AG hides under
                        # the fw node phase)
                        stq = fw_stq[k + 1] if d == 0 else bw_stq[k + 1]
                        tabq = fw_tabq[k + 1] if d == 0 else bw_tabq[k + 1]
                        allgather(stq[:], tabq[:])
                    if k < K - 1 and ui == len(units) - 3:
                        # edge-phase bw-dep gathers depend only on the bwq
                        # AllGather (done mid-fw-phase): issue them before
                        # the last fw blocks so their data is resident when
                        # the edge phase starts.
                        epend = [edge_gb_gather(k, j) for j in range(EPF)]
                if k < K - 1:
                    pend = []
                    fpend = [edge_gf_gather(k, b) for b in range(FPF)]
                    for b in range(EB):
                        if b + EPF < EB:
                            epend.append(edge_gb_gather(k, b + EPF))
                        if b + FPF < EB:
                            fpend.append(edge_gf_gather(k, b + FPF))
                        edge_block(k, b, epend[b], fpend[b])
                        # bf16 node tables are only needed by hop k+1's
                        # adjacency gathers: all-gather them during the edge
                        # phase, behind the critical fp8 AllGathers.
                        if b == 0:
                            allgather(bw_st[k + 1][:], bw_tabs[k + 1][:])
                        elif b == 1:
                            allgather(fw_st[k + 1][:], fw_tabs[k + 1][:])
                        elif b == ES_LO // P - 1:
                            # lo part (5/8) of the edge table: AllGather
                            # overlaps the remaining edge blocks; the exposed
                            # hi AllGather at the hop boundary shrinks.
                            dst = e_tabs[k + 1][0][:].rearrange(
                                "(c r) d -> c r d", r=ES_LO)
                            allgather(e_stq[k + 1][0:ES_LO, :], dst)
                    pend = [prefetch_unit(k + 1, *units[j], with_lo=(j < 2))
                            for j in range(pf)]
                    dst = e_tabs[k + 1][1][:].rearrange("(c r) d -> c r d",
                                                        r=ES_HI)
                    allgather(e_stq[k + 1][ES_LO:ES, :], dst)

    # Rebind SWDGE queue_num to the scheduled DMASW lane so each completion
    # semaphore always fires from one queue (the tile scheduler reorders
    # Pool DMA instructions).
    from concourse.tile_sem_assignment import PROC_NAME_TO_IDX
    idx_to_proc = {v: k for k, v in PROC_NAME_TO_IDX.items()}
    for blk in nc.m.functions[0].blocks:
        for inst in blk.instructions:
            if (inst.engine == mybir.EngineType.Pool
                    and hasattr(inst, "queue_num")
                    and getattr(inst, "bass_scheduled_proc", None) is not None):
                pname = idx_to_proc.get(inst.bass_scheduled_proc, "")
                if isinstance(pname, str) and pname.startswith("DMASW"):
                    inst.queue_num = int(pname[5:]) % 4

    nc.compile()
    return nc


def _pack_idx(lst):
    """[m] int (m % 128 == 0) -> [128, m/16] int16 wrapped gather layout."""
    wrapped = lst.astype(np.int16).reshape(-1, 16).T
    return np.tile(wrapped, (8, 1))


def prep_inputs(cfg: Cfg, plan: Plan, inputs: dict):
    import ml_dtypes
    bf16 = ml_dtypes.bfloat16
    fp8 = ml_dtypes.float8_e4m3
    N, E, D = cfg.N, cfg.E, cfg.D
    NS, ES, NB, EB, C = cfg.NS, cfg.ES, cfg.NB, cfg.EB, cfg.CORES
    KCN, KCE = cfg.KCN, cfg.KCE
    f32 = np.float32

    fw = np.asarray(inputs["fw_input"], f32)
    bw = np.asarray(inputs["bw_input"], f32)
    ee = np.asarray(inputs["edge_embs"], f32)
    adj = {0: np.asarray(inputs["fw_adj"], np.int64),
           1: np.asarray(inputs["bw_adj"], np.int64)}
    eid = {0: np.asarray(inputs["fw_edgeid"], np.int64),
           1: np.asarray(inputs["bw_edgeid"], np.int64)}
    dep = {0: np.asarray(inputs["fw_edgedep"], np.int64),
           1: np.asarray(inputs["bw_edgedep"], np.int64)}

    def wchunks(W, kc):
        W = np.asarray(W, f32)
        return np.concatenate([W[kk * P:(kk + 1) * P, :] for kk in range(kc)],
                              axis=1).astype(bf16)

    wfc = wchunks(inputs["Wfc"], KCN)
    wbc = wchunks(inputs["Wbc"], KCN)
    wed = wchunks(inputs["Wedge"], KCE)
    bfc = np.asarray(inputs["bfc"], f32).reshape(1, D).astype(bf16)
    bbc = np.asarray(inputs["bbc"], f32).reshape(1, D).astype(bf16)
    bed = np.asarray(inputs["bedge"], f32).reshape(1, D).astype(bf16)

    fw_tab0 = fw.astype(fp8)
    bw_tab0 = bw.astype(fp8)
    ES_LO = (ES * 5) // 8
    ES_HI = ES - ES_LO
    ee_r = ee.reshape(C, ES, D)
    e_tab0_lo = ee_r[:, :ES_LO].reshape(C * ES_LO, D).astype(fp8)
    e_tab0_hi = ee_r[:, ES_LO:].reshape(C * ES_HI, D).astype(fp8)

    def e_remap(v):
        # global edge id -> (half, row within half-table)
        cown = v // ES
        j = v % ES
        h = (j >= ES_LO).astype(np.int64)
        return h, np.where(h == 0, cown * ES_LO + j,
                           cown * ES_HI + (j - ES_LO))

    def pad_lists(vals, msk, kch, cnts):
        """valid list -> [kch*128]: valid entries, fake idx-0 fill up to
        each piece's shared count, -1 skip-tail beyond."""
        lst = vals[msk]
        m = kch * P
        lpad = np.full(m, -1, np.int64)
        lpad[:len(lst)] = lst
        off = 0
        for nch, cnt in zip(_pieces(kch), cnts):
            have = min(max(len(lst) - off * P, 0), nch * P)
            lpad[off * P + have: off * P + cnt] = 0
            off += nch
        return lpad

    in_maps = []
    for c in range(C):
        idx_cols = []
        dest_n = np.full((P, plan.n_chunks), -1.0, f32)
        rcn = np.zeros((P, 2 * NB), f32)
        for d in (0, 1):
            for b in range(NB):
                r0 = c * NS + b * P
                ka = int(plan.Ka[d, b])
                co = int(plan.n_off[d, b])
                av = adj[d][r0:r0 + P]
                ev = eid[d][r0:r0 + P]
                am, em = av >= 0, ev >= 0
                rcn[:, d * NB + b] = 1.0 / (am.sum(1) + em.sum(1))
                eh, erow = e_remap(np.maximum(ev, 0))
                eh = np.where(em, eh, -1)
                ke0 = int(plan.Ke[0, d, b])
                for (vals, msk, kch, base, cnts) in (
                        (av, am, ka, co, plan.cnt_a[d, b]),
                        (erow, eh == 0, ke0, co + ka, plan.cnt_e[0, d, b]),
                        (erow, eh == 1, int(plan.Ke[1, d, b]), co + ka + ke0,
                         plan.cnt_e[1, d, b])):
                    pidx, _ = np.nonzero(msk)
                    lst = vals[msk]
                    lpad = pad_lists(vals, msk, kch, cnts)
                    idx_cols.append(_pack_idx(lpad))
                    i = np.arange(len(lst))
                    dest_n[i % P, base + i // P] = pidx
        idx_n = np.concatenate(idx_cols, axis=1)

        idx_cols = []
        dest_e = np.full((P, plan.e_chunks), -1.0, f32)
        rcef = np.zeros((P, EB), f32)
        rceb = np.zeros((P, EB), f32)
        for b in range(EB):
            r0 = c * ES + b * P
            kf, kb = int(plan.Kf[b]), int(plan.Kb[b])
            co = int(plan.e_off[b])
            fv, bv = dep[0][r0:r0 + P], dep[1][r0:r0 + P]
            fm, bm = fv >= 0, bv >= 0
            rcef[:, b] = 1.0 / fm.sum(1)
            rceb[:, b] = 1.0 / bm.sum(1)
            for (vals, msk, kch, base, cnts) in (
                    (fv, fm, kf, 0, plan.cnt_f[b]),
                    (bv, bm, kb, kf, plan.cnt_b[b])):
                pidx, _ = np.nonzero(msk)
                lst = vals[msk]
                lpad = pad_lists(vals, msk, kch, cnts)
                idx_cols.append(_pack_idx(lpad))
                i = np.arange(len(lst))
                dest_e[i % P, co + base + i // P] = pidx
        idx_e = np.concatenate(idx_cols, axis=1)

        im = {
            "fw_tab0": fw_tab0, "bw_tab0": bw_tab0,
            "e_tab0_lo": e_tab0_lo, "e_tab0_hi": e_tab0_hi,
            "fw_own0": fw[c * NS:(c + 1) * NS].astype(bf16),
            "bw_own0": bw[c * NS:(c + 1) * NS].astype(bf16),
            "e_own0": ee[c * ES:(c + 1) * ES].astype(bf16),
            "idx_n": idx_n, "idx_e": idx_e,
            "dest_n": dest_n.astype(bf16), "dest_e": dest_e.astype(bf16),
            "rcn": rcn, "rcef": rcef, "rceb": rceb,
            "wfc": wfc, "wbc": wbc, "wed": wed,
            "bfc": bfc, "bbc": bbc, "bed": bed,
        }
        in_maps.append(im)
    return in_maps


def assemble_outputs(cfg: Cfg, results):
    fw = np.concatenate([results[c]["fw_out"] for c in range(cfg.CORES)], axis=0)
    bw = np.concatenate([results[c]["bw_out"] for c in range(cfg.CORES)], axis=0)
    return fw, bw


# ======================= self-contained runner =======================
import os as _os
import types as _types


def _install_axon_prof():
    name = "antenv.axon_hooks"
    if name in sys.modules:
        return True
    try:
        mod = _types.ModuleType(name)
        mod._hook = None
        mod.set_axon_ntff_profile_hook = lambda h: setattr(mod, "_hook", h)
        mod.get_axon_ntff_profile_hook = lambda: mod._hook
        sys.modules[name] = mod
        import antenv
        antenv.axon_hooks = mod
        from trn_agent_boot.trn_boot import _ntff_profile_via_ctypes
        mod.set_axon_ntff_profile_hook(
            _ntff_profile_via_ctypes('/opt/axon/libaxon_pjrt.so'))
        return True
    except Exception:
        sys.modules.pop(name, None)
        return False


_CACHE = {}
LAST_EXEC_NS = None
LAST_PROFILE = None


def kernel(**inputs):
    """Full-input GNN forward on 8 TRN2 NeuronCores. Returns (fw, bw)."""
    global LAST_EXEC_NS, LAST_PROFILE
    from concourse.bass_utils import run_bass_kernel_spmd

    cfg = Cfg()
    plan = Plan(cfg, inputs)
    key = plan.sig
    if _CACHE.get("key") != key:
        _CACHE["nc"] = build(cfg, plan)
        _CACHE["key"] = key
    nc = _CACHE["nc"]

    in_maps = prep_inputs(cfg, plan, inputs)

    profile = _os.environ.get("GNN_PROFILE", "0") == "1"
    if profile:
        profile = _install_axon_prof()
    res = run_bass_kernel_spmd(nc, in_maps, core_ids=list(range(cfg.CORES)),
                               trace=profile)
    LAST_EXEC_NS = res.exec_time_ns
    LAST_PROFILE = res.profile_json
    if res.instructions_and_trace is not None:
        try:
            print("trace:", res.instructions_and_trace[1])
        except Exception:
            pass
    return assemble_outputs(cfg, res.results)


# revision 50
# speedup vs baseline: 1.0013x; 1.0013x over previous
"""GNN message-passing kernel for TRN2, 8-core SPMD (self-contained).

v4 design (on top of v3), ~1.3 ms vs the 2.17 ms v3 baseline:
- Node rows sharded 8 ways (NS=N/8), edge rows too (ES=E/8).
- Mixed-precision gathers: node-adjacency gathers are bf16 at hops 1-2 and
  fp8-e4m3 at hop 0 (the hop-0 table is a host-quantized input, so no AG
  cost); edge-embedding gathers (node phase) and node-dep gathers (edge
  phase) are fp8 everywhere, halving their DMA bytes. CPU-sim rel err of
  this split 1.25e-2, HW 1.27e-2 (< 2e-2 gate).
- fp8 selector matmuls run pairwise in DoubleRow perf mode (2 fp8 weights
  per PE cell): ~2 chunks per 239 ns instead of 2x370 ns.
- Gather counts are compile-time per-piece maxima across cores; shorter
  cores pad with fake idx-0/dest=-1 entries, and the index tail beyond the
  shared count is -1 (SWDGE skips negative tails entirely). Gather-ring
  SBUF is memset once at startup so skipped tails can never feed NaNs into
  the zero-selector matmuls.
- AllGather restructure: small fp8 node tables (needed by the very next
  edge phase) gather right behind the producing node blocks; bf16 node
  tables (needed only by the NEXT hop's adjacency gathers) gather during
  the edge phase, off the critical path. The edge table is split 5/8 : 3/8
  into lo / hi part-tables (separate Shared tensors, host-remapped
  indices): the lo AllGather hides under the remaining edge blocks and the
  smaller hi AllGather shortens the exposed hop-boundary tail; the next
  hop's adjacency + lo-part gathers are prefetched before the hi AllGather
  so they run during it.
- Segment-mean via selector matmuls on the Tensor engine (is_equal-built
  0/1 selectors on the DVE), 1/cnt folded into the PSUM->SBUF activation
  copy. Linear layers bf16, bias via rank-1 matmul, ReLU fused in the
  PSUM->SBUF copy.
"""
import sys

sys.path.insert(0, '/opt/trn_rl_repo')

import numpy as np
import concourse.bass as bass
import concourse.mybir as mybir
from concourse import tile
from concourse.bacc import Bacc
from concourse.masks import make_identity

F32 = mybir.dt.float32
I32 = mybir.dt.int32
BF16 = mybir.dt.bfloat16
FP8 = mybir.dt.float8e4
I16 = mybir.dt.int16
P = 128

CMAX = 8  # max 128-row chunks per dma_gather call (ring capacity)


class Cfg:
    def __init__(self, N=8192, E=32768, D=512, DEG=16, DEP=8, K=3, CORES=8):
        self.N, self.E, self.D = N, E, D
        self.DEG, self.DEP, self.K, self.CORES = DEG, DEP, K, CORES
        self.NS = N // CORES
        self.ES = E // CORES
        self.NB = self.NS // P
        self.EB = self.ES // P
        self.DC = D // P
        self.KCN = (2 * D) // P
        self.KCE = (3 * D) // P
        assert self.NS % P == 0 and self.ES % P == 0 and D % P == 0
        assert 2 * N <= 32768 and E <= 32768  # int16 dma_gather indices


def _ceil128(x):
    return -(-x // 128)


def _pieces(n):
    out = []
    off = 0
    while off < n:
        out.append(min(CMAX, n - off))
        off += CMAX
    return out


class Plan:
    """Host-derived compile-time structure (chunk counts, column offsets),
    maxed across cores so one SPMD program fits all cores."""

    def __init__(self, cfg, inputs):
        NS, ES, NB, EB = cfg.NS, cfg.ES, cfg.NB, cfg.EB
        C = cfg.CORES
        adj = {0: np.asarray(inputs["fw_adj"], np.int64),
               1: np.asarray(inputs["bw_adj"], np.int64)}
        eid = {0: np.asarray(inputs["fw_edgeid"], np.int64),
               1: np.asarray(inputs["bw_edgeid"], np.int64)}
        dep = {0: np.asarray(inputs["fw_edgedep"], np.int64),
               1: np.asarray(inputs["bw_edgedep"], np.int64)}

        ES_LO = (ES * 5) // 8

        def e_half(v):
            # edge id -> which half-table it lives in (-1 for padding)
            return np.where(v < 0, -1, ((v % ES) >= ES_LO).astype(np.int64))

        self.Ka = np.zeros((2, NB), np.int64)
        self.Ke = np.zeros((2, 2, NB), np.int64)  # [half, d, b]
        self.Kf = np.zeros(EB, np.int64)
        self.Kb = np.zeros(EB, np.int64)
        for d in (0, 1):
            for b in range(NB):
                for c in range(C):
                    r0 = c * NS + b * P
                    self.Ka[d, b] = max(self.Ka[d, b],
                                        _ceil128((adj[d][r0:r0 + P] >= 0).sum()))
                    eh = e_half(eid[d][r0:r0 + P])
                    for h in (0, 1):
                        self.Ke[h, d, b] = max(self.Ke[h, d, b],
                                               _ceil128((eh == h).sum()))
        for b in range(EB):
            for c in range(C):
                r0 = c * ES + b * P
                self.Kf[b] = max(self.Kf[b], _ceil128((dep[0][r0:r0 + P] >= 0).sum()))
                self.Kb[b] = max(self.Kb[b], _ceil128((dep[1][r0:r0 + P] >= 0).sum()))

        self.n_off = np.zeros((2, NB), np.int64)
        off = 0
        for d in (0, 1):
            for b in range(NB):
                self.n_off[d, b] = off
                off += self.Ka[d, b] + self.Ke[0, d, b] + self.Ke[1, d, b]
        self.n_chunks = off
        self.e_off = np.zeros(EB, np.int64)
        off = 0
        for b in range(EB):
            self.e_off[b] = off
            off += self.Kf[b] + self.Kb[b]
        self.e_chunks = off

        # Per-piece transfer counts: max over cores of the piece's valid
        # count (compile-time constants; shorter cores pad with fake idx-0 /
        # dest=-1 entries up to the max, -1 skip-tail beyond). Keyed by
        # (kind, d_or_none, b, piece_idx) in issue order per block.
        def counts(vals_by_core, kch):
            per_core = [int((v >= 0).sum()) for v in vals_by_core]
            cnts = []
            off = 0
            for nch in _pieces(kch):
                c = max(min(max(vc - off * 128, 0), nch * 128)
                        for vc in per_core)
                cnts.append(max(c, 1))
                off += nch
            return cnts

        self.cnt_a = {}
        self.cnt_e = {}
        for d in (0, 1):
            for b in range(NB):
                rows = [adj[d][c * NS + b * P: c * NS + (b + 1) * P]
                        for c in range(C)]
                self.cnt_a[d, b] = counts(rows, int(self.Ka[d, b]))
                for h in (0, 1):
                    rows = [np.where(
                        e_half(eid[d][c * NS + b * P: c * NS + (b + 1) * P])
                        == h, 0, -1) for c in range(C)]
                    self.cnt_e[h, d, b] = counts(rows, int(self.Ke[h, d, b]))
        self.cnt_f = {}
        self.cnt_b = {}
        for b in range(EB):
            rows = [dep[0][c * ES + b * P: c * ES + (b + 1) * P]
                    for c in range(C)]
            self.cnt_f[b] = counts(rows, int(self.Kf[b]))
            rows = [dep[1][c * ES + b * P: c * ES + (b + 1) * P]
                    for c in range(C)]
            self.cnt_b[b] = counts(rows, int(self.Kb[b]))

        self.sig = (tuple(self.Ka.ravel()), tuple(self.Ke.ravel()),
                    tuple(self.Kf), tuple(self.Kb),
                    tuple(tuple(v) for v in self.cnt_a.values()),
                    tuple(tuple(v) for v in self.cnt_e.values()),
                    tuple(tuple(v) for v in self.cnt_f.values()),
                    tuple(tuple(v) for v in self.cnt_b.values()))


def build(cfg: Cfg, plan: Plan, pf=4):
    N, E, D = cfg.N, cfg.E, cfg.D
    K, CORES = cfg.K, cfg.CORES
    NS, ES, NB, EB = cfg.NS, cfg.ES, cfg.NB, cfg.EB
    DC, KCN, KCE = cfg.DC, cfg.KCN, cfg.KCE
    Ka, Ke, Kf, Kb = plan.Ka, plan.Ke, plan.Kf, plan.Kb
    KA_MAX = min(int(Ka.max()), CMAX)
    KE_MAX = min(int(Ke.max()), CMAX)
    KD_MAX = min(int(max(Kf.max(), Kb.max())), CMAX)
    SELA_MAX = int(Ka.max())
    SELE_MAX = int((Ke[0] + Ke[1]).max())
    SELD_MAX = int((Kf + Kb).max())
    ES_LO = (cfg.ES * 5) // 8
    ES_HI = cfg.ES - ES_LO
    E_LO = CORES * ES_LO
    E_HI = CORES * ES_HI

    nc = Bacc("TRN2", target_bir_lowering=False, debug=False, num_devices=CORES,
              num_swdge_queues=4)

    # ---- external inputs ----
    fw_tab0 = nc.dram_tensor("fw_tab0", [N, D], FP8, kind="ExternalInput")
    bw_tab0 = nc.dram_tensor("bw_tab0", [N, D], FP8, kind="ExternalInput")
    e_tab0_lo = nc.dram_tensor("e_tab0_lo", [E_LO, D], FP8, kind="ExternalInput")
    e_tab0_hi = nc.dram_tensor("e_tab0_hi", [E_HI, D], FP8, kind="ExternalInput")
    fw_own0 = nc.dram_tensor("fw_own0", [NS, D], BF16, kind="ExternalInput")
    bw_own0 = nc.dram_tensor("bw_own0", [NS, D], BF16, kind="ExternalInput")
    e_own0 = nc.dram_tensor("e_own0", [ES, D], BF16, kind="ExternalInput")
    idx_n = nc.dram_tensor("idx_n", [P, plan.n_chunks * 8], I16, kind="ExternalInput")
    idx_e = nc.dram_tensor("idx_e", [P, plan.e_chunks * 8], I16, kind="ExternalInput")
    dest_n = nc.dram_tensor("dest_n", [P, plan.n_chunks], BF16, kind="ExternalInput")
    dest_e = nc.dram_tensor("dest_e", [P, plan.e_chunks], BF16, kind="ExternalInput")
    rcn_x = nc.dram_tensor("rcn", [P, 2 * NB], F32, kind="ExternalInput")
    rcef_x = nc.dram_tensor("rcef", [P, EB], F32, kind="ExternalInput")
    rceb_x = nc.dram_tensor("rceb", [P, EB], F32, kind="ExternalInput")
    wfc_x = nc.dram_tensor("wfc", [P, KCN * D], BF16, kind="ExternalInput")
    wbc_x = nc.dram_tensor("wbc", [P, KCN * D], BF16, kind="ExternalInput")
    wed_x = nc.dram_tensor("wed", [P, KCE * D], BF16, kind="ExternalInput")
    bfc_x = nc.dram_tensor("bfc", [1, D], BF16, kind="ExternalInput")
    bbc_x = nc.dram_tensor("bbc", [1, D], BF16, kind="ExternalInput")
    bed_x = nc.dram_tensor("bed", [1, D], BF16, kind="ExternalInput")
    fw_out = nc.dram_tensor("fw_out", [NS, D], F32, kind="ExternalOutput")
    bw_out = nc.dram_tensor("bw_out", [NS, D], F32, kind="ExternalOutput")

    rg = [list(range(CORES))]
    RELU = mybir.ActivationFunctionType.Relu
    COPY = mybir.ActivationFunctionType.Copy
    EQ = mybir.AluOpType.is_equal

    with tile.TileContext(nc) as tc:
        with (
            tc.tile_pool(name="const", bufs=1) as cp,
            tc.tile_pool(name="gp", bufs=2) as gp,
            tc.tile_pool(name="slp", bufs=3) as slp,
            tc.tile_pool(name="sp", bufs=3) as sp,
            tc.tile_pool(name="xp", bufs=2) as xp,
            tc.tile_pool(name="op", bufs=3) as op,
            tc.tile_pool(name="pm", bufs=2, space="PSUM") as pmp,
            tc.tile_pool(name="pt", bufs=1, space="PSUM") as ptp,
            tc.tile_pool(name="po", bufs=2, space="PSUM") as pop,
            tc.tile_pool(name="dram", bufs=1, space="DRAM") as dp,
        ):
            # ---- constants ----
            ident = cp.tile([P, P], BF16)
            make_identity(nc, ident[:])
            ones1 = cp.tile([1, P], BF16)
            nc.gpsimd.memset(ones1[:], 1.0)
            iota_i = cp.tile([P, P], I32)
            nc.gpsimd.iota(iota_i[:], pattern=[[1, P]], base=0,
                           channel_multiplier=0)
            iota_b = cp.tile([P, P], BF16)
            nc.vector.tensor_copy(out=iota_b[:], in_=iota_i[:])

            def load_flat(name, src, shape, dt):
                t = cp.tile(shape, dt, name=name)
                nc.sync.dma_start(out=t[:], in_=src[:])
                return t

            ixn_t = load_flat("ixn_t", idx_n, [P, plan.n_chunks * 8], I16)
            dn_t = load_flat("dn_t", dest_n, [P, plan.n_chunks], BF16)
            rcn_t = load_flat("rcn_t", rcn_x, [P, 2 * NB], F32)
            ixe_t = load_flat("ixe_t", idx_e, [P, plan.e_chunks * 8], I16)
            de_t = load_flat("de_t", dest_e, [P, plan.e_chunks], BF16)
            wfc_t = load_flat("wfc_t", wfc_x, [P, KCN * D], BF16)
            wbc_t = load_flat("wbc_t", wbc_x, [P, KCN * D], BF16)
            wed_t = load_flat("wed_t", wed_x, [P, KCE * D], BF16)
            bfc_t = load_flat("bfc_t", bfc_x, [1, D], BF16)
            bbc_t = load_flat("bbc_t", bbc_x, [1, D], BF16)
            bed_t = load_flat("bed_t", bed_x, [1, D], BF16)
            rcef_t = load_flat("rcef_t", rcef_x, [P, EB], F32)
            rceb_t = load_flat("rceb_t", rceb_x, [P, EB], F32)

            # ---- DRAM tables / staging ----
            fw_tabs = [fw_tab0] + [dp.tile([N, D], BF16, addr_space="Shared",
                                           name=f"fw_tab{k}") for k in (1, 2)]
            bw_tabs = [bw_tab0] + [dp.tile([N, D], BF16, addr_space="Shared",
                                           name=f"bw_tab{k}") for k in (1, 2)]
            n_tabs = [(fw_tabs[k], bw_tabs[k]) for k in range(K)]
            # per-direction fp8 node tables (edge-phase dep gathers): the
            # bw AllGather triggers right after the bw node blocks and hides
            # under the fw node phase, so edge gb gathers start immediately.
            fw_tabq = [None] + [dp.tile([N, D], FP8, addr_space="Shared",
                                        name=f"fw_tq{k}") for k in (1, 2)]
            bw_tabq = [None] + [dp.tile([N, D], FP8, addr_space="Shared",
                                        name=f"bw_tq{k}") for k in (1, 2)]
            e_tabs = [(e_tab0_lo, e_tab0_hi)] + [
                (dp.tile([E_LO, D], FP8, addr_space="Shared", name=f"e_tl{k}"),
                 dp.tile([E_HI, D], FP8, addr_space="Shared", name=f"e_th{k}"))
                for k in (1, 2)]
            fw_st = [fw_own0, dp.tile([NS, D], BF16, name="fw_shA"),
                     dp.tile([NS, D], BF16, name="fw_shB")]
            bw_st = [bw_own0, dp.tile([NS, D], BF16, name="bw_shA"),
                     dp.tile([NS, D], BF16, name="bw_shB")]
            fw_stq = [None, dp.tile([NS, D], FP8, name="fw_qA"),
                      dp.tile([NS, D], FP8, name="fw_qB")]
            bw_stq = [None, dp.tile([NS, D], FP8, name="bw_qA"),
                      dp.tile([NS, D], FP8, name="bw_qB")]
            e_st = [e_own0, dp.tile([ES, D], BF16, name="e_shA"),
                    dp.tile([ES, D], BF16, name="e_shB")]
            e_stq = [None, dp.tile([ES, D], FP8, name="e_qA"),
                     dp.tile([ES, D], FP8, name="e_qB")]

            qctr = [0]

            def gather(tab_ap, idx_tile, chunk_off, nchunks, tag, maxch, dt,
                       cnts, bufs=None, into=None, into_col=0,
                       full_cnt=False):
                """ceil(nchunks/CMAX) dma_gather calls -> [(tile, col, nch)].
                cnts[i] = compile-time transfer count (max across cores).
                into/into_col: write into an existing tile at a chunk col."""
                out = []
                off = 0
                pi = 0
                while off < nchunks:
                    nch = min(CMAX, nchunks - off)
                    if into is None:
                        g = gp.tile([P, min(maxch, CMAX) * D], dt,
                                    name=f"g_{tag}", tag=tag, bufs=bufs)
                        col = 0
                    else:
                        g = into
                        col = into_col + off
                    qctr[0] = (qctr[0] + 1) % 4
                    nc.gpsimd.dma_gather(
                        out_ap=g[:, col * D:(col + nch) * D]
                            .rearrange("p (t e) -> p t e", e=D),
                        in_ap=tab_ap,
                        idxs_ap=idx_tile[:, (chunk_off + off) * 8:
                                         (chunk_off + off + nch) * 8],
                        num_idxs=nch * P,
                        num_idxs_reg=nch * P if full_cnt else int(cnts[pi]),
                        elem_size=D,
                        queue_num=qctr[0],
                    )
                    out.append((g, col, nch))
                    off += nch
                    pi += 1
                return out

            def allgather(src_ap, dst_ap):
                nc.gpsimd.collective_compute(
                    "AllGather", mybir.AluOpType.bypass, replica_groups=rg,
                    ins=[src_ap], outs=[dst_ap],
                )

            def ag_rows(st, tab, rows_total, r0, r1):
                """AllGather staging rows [r0:r1) into the strided full-table
                view [C, rows_total, D][:, r0:r1, :]."""
                dst = tab[:].rearrange("(c r) d -> c r d", r=rows_total)
                allgather(st[r0:r1, :], dst[:, r0:r1, :])

            def build_sel(dtile, co, nch, dt, tag, smax):
                """[128, nch*128] selector: sel[r, c*128+p] =
                (dest[r, co+c] == p)."""
                st = slp.tile([P, smax * P], dt, name=f"sel_{tag}", tag=tag)
                io_b = iota_b[:].rearrange("p (o f) -> p o f", o=1) \
                                .broadcast_to([P, nch, P])
                db = dtile[:, co:co + nch].rearrange("p (c o) -> p c o", o=1) \
                                          .broadcast_to([P, nch, P])
                nc.vector.tensor_tensor(
                    out=st[:, :nch * P].rearrange("p (c f) -> p c f", f=P),
                    in0=io_b, in1=db, op=EQ)
                return st

            def flat_chunks(glist):
                return [(g, col + c) for g, col, n in glist for c in range(n)]

            DR = mybir.MatmulPerfMode.DoubleRow

            def sel_matmul(ps, sel_t, c0, chunks, first, last):
                # pair adjacent fp8 chunks from the same gather tile into
                # DoubleRow matmuls (2 fp8 weights per PE cell)
                groups = []
                i = 0
                while i < len(chunks):
                    g, c = chunks[i]
                    if (sel_t.dtype == FP8 and i + 1 < len(chunks)
                            and chunks[i + 1][0] is g
                            and chunks[i + 1][1] == c + 1):
                        groups.append((g, c, i, True))
                        i += 2
                    else:
                        groups.append((g, c, i, False))
                        i += 1
                for gi, (g, c, i, dbl) in enumerate(groups):
                    st = first and gi == 0
                    sp_ = last and gi == len(groups) - 1
                    if dbl:
                        nc.tensor.matmul(
                            out=ps,
                            lhsT=sel_t[:, (c0 + i) * P:(c0 + i + 2) * P]
                                .rearrange("p (k m) -> p k m", k=2),
                            rhs=g[:, c * D:(c + 2) * D]
                                .rearrange("p (k d) -> p k d", k=2),
                            start=st, stop=sp_, perf_mode=DR,
                        )
                    else:
                        nc.tensor.matmul(
                            out=ps,
                            lhsT=sel_t[:, (c0 + i) * P:(c0 + i + 1) * P],
                            rhs=g[:, c * D:(c + 1) * D],
                            start=st, stop=sp_,
                        )

            def transpose_into(pt, cbase, src_sb, nch):
                for c in range(nch):
                    nc.tensor.transpose(
                        out=pt[:, (cbase + c) * P:(cbase + c + 1) * P],
                        in_=src_sb[:, c * P:(c + 1) * P],
                        identity=ident[:],
                    )

            def linear(xT, kc, w_t, b_row):
                ps = pop.tile([P, D], F32, name="ps_o", tag="ps_o")
                for kk in range(kc):
                    nc.tensor.matmul(
                        out=ps[:], lhsT=xT[:, kk * P:(kk + 1) * P],
                        rhs=w_t[:, kk * D:(kk + 1) * D],
                        start=(kk == 0), stop=False,
                    )
                nc.tensor.matmul(out=ps[:], lhsT=ones1[:], rhs=b_row[:],
                                 start=False, stop=True)
                return ps

            GA_BUFS = 2 * (pf + 1)

            # Zero every gather-ring buffer once: skipped -1 tails leave
            # stale SBUF that the selector matmuls read (zero-selector), and
            # uninitialized bits could decode as NaN (0 * NaN = NaN).
            for i in range(GA_BUFS):
                t = gp.tile([P, min(KA_MAX, CMAX) * D], BF16, name="z_ga",
                            tag="ga", bufs=GA_BUFS)
                eng = nc.vector if i % 2 == 0 else nc.gpsimd
                eng.memset(t[:], 0.0)
            for i in range(4):
                t = gp.tile([P, min(KE_MAX, CMAX) * D], FP8, name="z_ge",
                            tag="ge", bufs=4)
                eng = nc.gpsimd if i % 2 == 0 else nc.vector
                eng.memset(t[:], 0.0)

            def node_adj_gather(k, d, b):
                tab = n_tabs[k][d]
                dt = FP8 if k == 0 else BF16
                return gather(tab[:], ixn_t, int(plan.n_off[d, b]),
                              int(Ka[d, b]), "ga", KA_MAX, dt,
                              plan.cnt_a[d, b], bufs=GA_BUFS)

            def node_ge_lo(k, d, b):
                ke0 = int(Ke[0, d, b])
                co = int(plan.n_off[d, b]) + int(Ka[d, b])
                lo = gather(e_tabs[k][0][:], ixn_t, co, ke0, "ge", KE_MAX,
                            FP8, plan.cnt_e[0, d, b], bufs=4)
                return True, lo

            def node_ge_hi(k, d, b, gt):
                ke0, ke1 = int(Ke[0, d, b]), int(Ke[1, d, b])
                co = int(plan.n_off[d, b]) + int(Ka[d, b])
                return gather(e_tabs[k][1][:], ixn_t, co + ke0, ke1, "ge",
                              KE_MAX, FP8, plan.cnt_e[1, d, b], bufs=4)

            def node_block(k, d, b, ga, ge):
                last = (k == K - 1)
                ka = int(Ka[d, b])
                ke0, ke1 = int(Ke[0, d, b]), int(Ke[1, d, b])
                ke = ke0 + ke1
                co = int(plan.n_off[d, b])
                adt = FP8 if k == 0 else BF16
                sel_a = build_sel(dn_t, co, ka, adt, "sela", SELA_MAX)
                sel_e = build_sel(dn_t, co + ka, ke, FP8, "sele", SELE_MAX)
                own = sp.tile([P, D], BF16, name="own", tag="own")
                st = fw_st[k] if d == 0 else bw_st[k]
                nc.sync.dma_start(out=own[:], in_=st[b * P:(b + 1) * P, :])

                ps_m = pmp.tile([P, D], F32, name="ps_m", tag="ps_f")
                sel_matmul(ps_m[:], sel_a, 0, flat_chunks(ga),
                           True, ke == 0)
                sel_matmul(ps_m[:], sel_e, 0, flat_chunks(ge),
                           ka == 0, True)
                m_sb = sp.tile([P, D], BF16, name="m_sb", tag="m")
                nc.scalar.activation(out=m_sb[:], in_=ps_m[:], func=COPY,
                                     scale=rcn_t[:, d * NB + b:d * NB + b + 1])

                pt = ptp.tile([P, KCN * P], BF16, name="pt", tag="pt")
                transpose_into(pt, 0, own[:], DC)
                transpose_into(pt, DC, m_sb[:], DC)
                xT = xp.tile([P, KCN * P], BF16, name="xT", tag="xT")
                nc.vector.tensor_copy(out=xT[:], in_=pt[:])

                w_t = wfc_t if d == 0 else wbc_t
                b_row = bfc_t if d == 0 else bbc_t
                ps_o = linear(xT, KCN, w_t, b_row)
                if not last:
                    ob = op.tile([P, D], BF16, name="ob", tag="ob")
                    nc.scalar.activation(out=ob[:], in_=ps_o[:], func=RELU)
                    obq = op.tile([P, D], FP8, name="obq", tag="obq")
                    nc.scalar.activation(out=obq[:], in_=ps_o[:], func=RELU)
                    dst = fw_st[k + 1] if d == 0 else bw_st[k + 1]
                    dstq = fw_stq[k + 1] if d == 0 else bw_stq[k + 1]
                    nc.sync.dma_start(out=dst[b * P:(b + 1) * P, :], in_=ob[:])
                    nc.sync.dma_start(out=dstq[b * P:(b + 1) * P, :],
                                      in_=obq[:])
                else:
                    of = op.tile([P, D], F32, name="of", tag="of")
                    nc.scalar.activation(out=of[:], in_=ps_o[:], func=COPY)
                    dst = fw_out if d == 0 else bw_out
                    nc.sync.dma_start(out=dst[b * P:(b + 1) * P, :], in_=of[:])

            def edge_gb_gather(k, b):
                kf, kb = int(Kf[b]), int(Kb[b])
                co = int(plan.e_off[b])
                return gather(bw_tabq[k + 1][:], ixe_t, co + kf, kb, "gd",
                              KD_MAX, FP8, plan.cnt_b[b], bufs=8)

            def edge_gf_gather(k, b):
                kf = int(Kf[b])
                co = int(plan.e_off[b])
                return gather(fw_tabq[k + 1][:], ixe_t, co, kf, "gd",
                              KD_MAX, FP8, plan.cnt_f[b], bufs=8)

            def edge_block(k, b, gb, gf):
                kf, kb = int(Kf[b]), int(Kb[b])
                co = int(plan.e_off[b])
                sel_t = build_sel(de_t, co, kf + kb, FP8, "seld", SELD_MAX)
                own = sp.tile([P, D], BF16, name="own_e", tag="own")
                nc.sync.dma_start(out=own[:],
                                  in_=e_st[k][b * P:(b + 1) * P, :])

                # bw half first, fw half second (independent PSUM tiles so
                # each half retires on its own).
                ps_b = pmp.tile([P, D], F32, name="ps_be", tag="ps_b")
                sel_matmul(ps_b[:], sel_t, kf, flat_chunks(gb),
                           True, True)
                mb = sp.tile([P, D], BF16, name="mb", tag="m2")
                nc.scalar.activation(out=mb[:], in_=ps_b[:], func=COPY,
                                     scale=rceb_t[:, b:b + 1])

                ps_f = pmp.tile([P, D], F32, name="ps_fe", tag="ps_f")
                sel_matmul(ps_f[:], sel_t, 0, flat_chunks(gf),
                           True, True)
                mf = sp.tile([P, D], BF16, name="mf", tag="m")
                nc.scalar.activation(out=mf[:], in_=ps_f[:], func=COPY,
                                     scale=rcef_t[:, b:b + 1])

                pt = ptp.tile([P, KCE * P], BF16, name="pt_e", tag="pt")
                transpose_into(pt, 0, own[:], DC)
                transpose_into(pt, DC, mf[:], DC)
                transpose_into(pt, 2 * DC, mb[:], DC)
                xT = xp.tile([P, KCE * P], BF16, name="xT_e", tag="xT")
                nc.vector.tensor_copy(out=xT[:], in_=pt[:])

                ps_o = linear(xT, KCE, wed_t, bed_t)
                eb = op.tile([P, D], BF16, name="eb", tag="ob")
                nc.scalar.activation(out=eb[:], in_=ps_o[:], func=RELU)
                ebq = op.tile([P, D], FP8, name="ebq", tag="obq")
                nc.scalar.activation(out=ebq[:], in_=ps_o[:], func=RELU)
                nc.sync.dma_start(out=e_st[k + 1][b * P:(b + 1) * P, :],
                                  in_=eb[:])
                nc.sync.dma_start(out=e_stq[k + 1][b * P:(b + 1) * P, :],
                                  in_=ebq[:])

            # ---------------- program ----------------
            def prefetch_unit(k, d, b, with_lo):
                ent = {"ga": node_adj_gather(k, d, b), "gt": None, "lo": None}
                if with_lo:
                    ent["gt"], ent["lo"] = node_ge_lo(k, d, b)
                return ent

            units = [(d, b) for d in (1, 0) for b in range(NB)]
            pend = [prefetch_unit(0, *units[j], with_lo=(j < 2))
                    for j in range(pf)]
            EPF = 6
            FPF = 3
            for k in range(K):
                epend = None
                for ui, (d, b) in enumerate(units):
                    if k == 0 and ui == 0:
                        # edge-gather ring is first touched in the edge
                        # phase: zero it during the node phase
                        for i in range(8):
                            t = gp.tile([P, min(KD_MAX, CMAX) * D], FP8,
                                        name="z_gd", tag="gd", bufs=8)
                            nc.vector.memset(t[:], 0.0)
                    ent = pend[ui]
                    if ui + pf < len(units):
                        pend.append(
                            prefetch_unit(k, *units[ui + pf], with_lo=False))
                    if ent["gt"] is None:
                        ent["gt"], ent["lo"] = node_ge_lo(k, d, b)
                    ge = ent["lo"] + node_ge_hi(k, d, b, ent["gt"])
                    node_block(k, d, b, ent["ga"], ge)
                    if k < K - 1 and b == NB - 1:
                        # per-direction fp8 AllGather right behind its last
                        # producing block (bw first, so its AG hides under
                        # the fw node phase)
                        stq = fw_stq[k + 1] if d == 0 else bw_stq[k + 1]
                        tabq = fw_tabq[k + 1] if d == 0 else bw_tabq[k + 1]
                        allgather(stq[:], tabq[:])
                    if k < K - 1 and ui == len(units) - 3:
                        # edge-phase bw-dep gathers depend only on the bwq
                        # AllGather (done mid-fw-phase): issue them before
                        # the last fw blocks so their data is resident when
                        # the edge phase starts.
                        epend = [edge_gb_gather(k, j) for j in range(EPF)]
                if k < K - 1:
                    pend = []
                    fpend = [edge_gf_gather(k, b) for b in range(FPF)]
                    for b in range(EB):
                        if b + EPF < EB:
                            epend.append(edge_gb_gather(k, b + EPF))
                        if b + FPF < EB:
                            fpend.append(edge_gf_gather(k, b + FPF))
                        edge_block(k, b, epend[b], fpend[b])
                        # bf16 node tables are only needed by hop k+1's
                        # adjacency gathers: all-gather them during the edge
                        # phase, behind the critical fp8 AllGathers.
                        if b == 0:
                            allgather(bw_st[k + 1][:], bw_tabs[k + 1][:])
                        elif b == 1:
                            allgather(fw_st[k + 1][:], fw_tabs[k + 1][:])
                        elif b == ES_LO // P - 1:
                            # lo part (5/8) of the edge table: AllGather
                            # overlaps the remaining edge blocks; the exposed
                            # hi AllGather at the hop boundary shrinks.
                            dst = e_tabs[k + 1][0][:].rearrange(
                                "(c r) d -> c r d", r=ES_LO)
                            allgather(e_stq[k + 1][0:ES_LO, :], dst)
                        elif b == EB - 5:
                            # next hop's adjacency gathers must be issued
                            # well before the e_hi trigger: the scheduler
                            # otherwise places them behind its Pool fence
                            # and they only run after the AllGather.
                            pend.append(prefetch_unit(k + 1, *units[0],
                                                      with_lo=False))
                            pend.append(prefetch_unit(k + 1, *units[1],
                                                      with_lo=False))
                        elif b == EB - 3:
                            pend.append(prefetch_unit(k + 1, *units[2],
                                                      with_lo=False))
                            pend.append(prefetch_unit(k + 1, *units[3],
                                                      with_lo=False))
                    # lo-part edge-emb gathers last: their e_lo AllGather
                    # dependency is only now safely complete (head-of-line)
                    for j in (0, 1):
                        pend[j]["gt"], pend[j]["lo"] = node_ge_lo(
                            k + 1, *units[j])
                    dst = e_tabs[k + 1][1][:].rearrange("(c r) d -> c r d",
                                                        r=ES_HI)
                    allgather(e_stq[k + 1][ES_LO:ES, :], dst)

    # Rebind SWDGE queue_num to the scheduled DMASW lane so each completion
    # semaphore always fires from one queue (the tile scheduler reorders
    # Pool DMA instructions).
    from concourse.tile_sem_assignment import PROC_NAME_TO_IDX
    idx_to_proc = {v: k for k, v in PROC_NAME_TO_IDX.items()}
    for blk in nc.m.functions[0].blocks:
        for inst in blk.instructions:
            if (inst.engine == mybir.EngineType.Pool
                    and hasattr(inst, "queue_num")
                    and getattr(inst, "bass_scheduled_proc", None) is not None):
                pname = idx_to_proc.get(inst.bass_scheduled_proc, "")
                if isinstance(pname, str) and pname.startswith("DMASW"):
                    inst.queue_num = int(pname[5:]) % 4

    nc.compile()
    return nc


def _pack_idx(lst):
    """[m] int (m % 128 == 0) -> [128, m/16] int16 wrapped gather layout."""
    wrapped = lst.astype(np.int16).reshape(-1, 16).T
    return np.tile(wrapped, (8, 1))


def prep_inputs(cfg: Cfg, plan: Plan, inputs: dict):
    import ml_dtypes
    bf16 = ml_dtypes.bfloat16
    fp8 = ml_dtypes.float8_e4m3
    N, E, D = cfg.N, cfg.E, cfg.D
    NS, ES, NB, EB, C = cfg.NS, cfg.ES, cfg.NB, cfg.EB, cfg.CORES
    KCN, KCE = cfg.KCN, cfg.KCE
    f32 = np.float32

    fw = np.asarray(inputs["fw_input"], f32)
    bw = np.asarray(inputs["bw_input"], f32)
    ee = np.asarray(inputs["edge_embs"], f32)
    adj = {0: np.asarray(inputs["fw_adj"], np.int64),
           1: np.asarray(inputs["bw_adj"], np.int64)}
    eid = {0: np.asarray(inputs["fw_edgeid"], np.int64),
           1: np.asarray(inputs["bw_edgeid"], np.int64)}
    dep = {0: np.asarray(inputs["fw_edgedep"], np.int64),
           1: np.asarray(inputs["bw_edgedep"], np.int64)}

    def wchunks(W, kc):
        W = np.asarray(W, f32)
        return np.concatenate([W[kk * P:(kk + 1) * P, :] for kk in range(kc)],
                              axis=1).astype(bf16)

    wfc = wchunks(inputs["Wfc"], KCN)
    wbc = wchunks(inputs["Wbc"], KCN)
    wed = wchunks(inputs["Wedge"], KCE)
    bfc = np.asarray(inputs["bfc"], f32).reshape(1, D).astype(bf16)
    bbc = np.asarray(inputs["bbc"], f32).reshape(1, D).astype(bf16)
    bed = np.asarray(inputs["bedge"], f32).reshape(1, D).astype(bf16)

    fw_tab0 = fw.astype(fp8)
    bw_tab0 = bw.astype(fp8)
    ES_LO = (ES * 5) // 8
    ES_HI = ES - ES_LO
    ee_r = ee.reshape(C, ES, D)
    e_tab0_lo = ee_r[:, :ES_LO].reshape(C * ES_LO, D).astype(fp8)
    e_tab0_hi = ee_r[:, ES_LO:].reshape(C * ES_HI, D).astype(fp8)

    def e_remap(v):
        # global edge id -> (half, row within half-table)
        cown = v // ES
        j = v % ES
        h = (j >= ES_LO).astype(np.int64)
        return h, np.where(h == 0, cown * ES_LO + j,
                           cown * ES_HI + (j - ES_LO))

    def pad_lists(vals, msk, kch, cnts):
        """valid list -> [kch*128]: valid entries, fake idx-0 fill up to
        each piece's shared count, -1 skip-tail beyond."""
        lst = vals[msk]
        m = kch * P
        lpad = np.full(m, -1, np.int64)
        lpad[:len(lst)] = lst
        off = 0
        for nch, cnt in zip(_pieces(kch), cnts):
            have = min(max(len(lst) - off * P, 0), nch * P)
            lpad[off * P + have: off * P + cnt] = 0
            off += nch
        return lpad

    in_maps = []
    for c in range(C):
        idx_cols = []
        dest_n = np.full((P, plan.n_chunks), -1.0, f32)
        rcn = np.zeros((P, 2 * NB), f32)
        for d in (0, 1):
            for b in range(NB):
                r0 = c * NS + b * P
                ka = int(plan.Ka[d, b])
                co = int(plan.n_off[d, b])
                av = adj[d][r0:r0 + P]
                ev = eid[d][r0:r0 + P]
                am, em = av >= 0, ev >= 0
                rcn[:, d * NB + b] = 1.0 / (am.sum(1) + em.sum(1))
                eh, erow = e_remap(np.maximum(ev, 0))
                eh = np.where(em, eh, -1)
                ke0 = int(plan.Ke[0, d, b])
                for (vals, msk, kch, base, cnts) in (
                        (av, am, ka, co, plan.cnt_a[d, b]),
                        (erow, eh == 0, ke0, co + ka, plan.cnt_e[0, d, b]),
                        (erow, eh == 1, int(plan.Ke[1, d, b]), co + ka + ke0,
                         plan.cnt_e[1, d, b])):
                    pidx, _ = np.nonzero(msk)
                    lst = vals[msk]
                    lpad = pad_lists(vals, msk, kch, cnts)
                    idx_cols.append(_pack_idx(lpad))
                    i = np.arange(len(lst))
                    dest_n[i % P, base + i // P] = pidx
        idx_n = np.concatenate(idx_cols, axis=1)

        idx_cols = []
        dest_e = np.full((P, plan.e_chunks), -1.0, f32)
        rcef = np.zeros((P, EB), f32)
        rceb = np.zeros((P, EB), f32)
        for b in range(EB):
            r0 = c * ES + b * P
            kf, kb = int(plan.Kf[b]), int(plan.Kb[b])
            co = int(plan.e_off[b])
            fv, bv = dep[0][r0:r0 + P], dep[1][r0:r0 + P]
            fm, bm = fv >= 0, bv >= 0
            rcef[:, b] = 1.0 / fm.sum(1)
            rceb[:, b] = 1.0 / bm.sum(1)
            for (vals, msk, kch, base, cnts) in (
                    (fv, fm, kf, 0, plan.cnt_f[b]),
                    (bv, bm, kb, kf, plan.cnt_b[b])):
                pidx, _ = np.nonzero(msk)
                lst = vals[msk]
                lpad = pad_lists(vals, msk, kch, cnts)
                idx_cols.append(_pack_idx(lpad))
                i = np.arange(len(lst))
                dest_e[i % P, co + base + i // P] = pidx
        idx_e = np.concatenate(idx_cols, axis=1)

        im = {
            "fw_tab0": fw_tab0, "bw_tab0": bw_tab0,
            "e_tab0_lo": e_tab0_lo, "e_tab0_hi": e_tab0_hi,
            "fw_own0": fw[c * NS:(c + 1) * NS].astype(bf16),
            "bw_own0": bw[c * NS:(c + 1) * NS].astype(bf16),
            "e_own0": ee[c * ES:(c + 1) * ES].astype(bf16),
            "idx_n": idx_n, "idx_e": idx_e,
            "dest_n": dest_n.astype(bf16), "dest_e": dest_e.astype(bf16),
            "rcn": rcn, "rcef": rcef, "rceb": rceb,
            "wfc": wfc, "wbc": wbc, "wed": wed,
            "bfc": bfc, "bbc": bbc, "bed": bed,
        }
        in_maps.append(im)
    return in_maps


def assemble_outputs(cfg: Cfg, results):
    fw = np.concatenate([results[c]["fw_out"] for c in range(cfg.CORES)], axis=0)
    bw = np.concatenate([results[c]["bw_out"] for c in range(cfg.CORES)], axis=0)
    return fw, bw


# ======================= self-contained runner =======================
import os as _os
import types as _types


def _install_axon_prof():
    name = "antenv.axon_hooks"
    if name in sys.modules:
        return True
    try:
        mod = _types.ModuleType(name)
        mod._hook = None
        mod.set_axon_ntff_profile_hook = lambda h: setattr(mod, "_hook", h)
        mod.get_axon_ntff_profile_hook = lambda: mod._hook
        sys.modules[name] = mod
        import antenv
        antenv.axon_hooks = mod
        from trn_agent_boot.trn_boot import _ntff_profile_via_ctypes
        mod.set_axon_ntff_profile_hook(
            _ntff_profile_via_ctypes('/opt/axon/libaxon_pjrt.so'))
        return True
    except Exception:
        sys.modules.pop(name, None)
        return False


_CACHE = {}
LAST_EXEC_NS = None
LAST_PROFILE = None


def kernel(**inputs):
    """Full-input GNN forward on 8 TRN2 NeuronCores. Returns (fw, bw)."""
    global LAST_EXEC_NS, LAST_PROFILE
    from concourse.bass_utils import run_bass_kernel_spmd

    cfg = Cfg()
    plan = Plan(cfg, inputs)
    key = plan.sig
    if _CACHE.get("key") != key:
        _CACHE["nc"] = build(cfg, plan)
        _CACHE["key"] = key
    nc = _CACHE["nc"]

    in_maps = prep_inputs(cfg, plan, inputs)

    profile = _os.environ.get("GNN_PROFILE", "0") == "1"
    if profile:
        profile = _install_axon_prof()
    res = run_bass_kernel_spmd(nc, in_maps, core_ids=list(range(cfg.CORES)),
                               trace=profile)
    LAST_EXEC_NS = res.exec_time_ns
    LAST_PROFILE = res.profile_json
    if res.instructions_and_trace is not None:
        try:
            print("trace:", res.instructions_and_trace[1])
        except Exception:
            pass
    return assemble_outputs(cfg, res.results)


# revision 51
# speedup vs baseline: 1.0246x; 1.0233x over previous
"""GNN message-passing kernel for TRN2, 8-core SPMD (self-contained).

v4 design (on top of v3), ~1.3 ms vs the 2.17 ms v3 baseline:
- Node rows sharded 8 ways (NS=N/8), edge rows too (ES=E/8).
- Mixed-precision gathers: node-adjacency gathers are bf16 at hops 1-2 and
  fp8-e4m3 at hop 0 (the hop-0 table is a host-quantized input, so no AG
  cost); edge-embedding gathers (node phase) and node-dep gathers (edge
  phase) are fp8 everywhere, halving their DMA bytes. CPU-sim rel err of
  this split 1.25e-2, HW 1.27e-2 (< 2e-2 gate).
- fp8 selector matmuls run pairwise in DoubleRow perf mode (2 fp8 weights
  per PE cell): ~2 chunks per 239 ns instead of 2x370 ns.
- Gather counts are compile-time per-piece maxima across cores; shorter
  cores pad with fake idx-0/dest=-1 entries, and the index tail beyond the
  shared count is -1 (SWDGE skips negative tails entirely). Gather-ring
  SBUF is memset once at startup so skipped tails can never feed NaNs into
  the zero-selector matmuls.
- AllGather restructure: small fp8 node tables (needed by the very next
  edge phase) gather right behind the producing node blocks; bf16 node
  tables (needed only by the NEXT hop's adjacency gathers) gather during
  the edge phase, off the critical path. The edge table is split 5/8 : 3/8
  into lo / hi part-tables (separate Shared tensors, host-remapped
  indices): the lo AllGather hides under the remaining edge blocks and the
  smaller hi AllGather shortens the exposed hop-boundary tail; the next
  hop's adjacency + lo-part gathers are prefetched before the hi AllGather
  so they run during it.
- Segment-mean via selector matmuls on the Tensor engine (is_equal-built
  0/1 selectors on the DVE), 1/cnt folded into the PSUM->SBUF activation
  copy. Linear layers bf16, bias via rank-1 matmul, ReLU fused in the
  PSUM->SBUF copy.
"""
import sys

sys.path.insert(0, '/opt/trn_rl_repo')

import numpy as np
import concourse.bass as bass
import concourse.mybir as mybir
from concourse import tile
from concourse.bacc import Bacc
from concourse.masks import make_identity

F32 = mybir.dt.float32
I32 = mybir.dt.int32
BF16 = mybir.dt.bfloat16
FP8 = mybir.dt.float8e4
I16 = mybir.dt.int16
P = 128

CMAX = 8  # max 128-row chunks per dma_gather call (ring capacity)


class Cfg:
    def __init__(self, N=8192, E=32768, D=512, DEG=16, DEP=8, K=3, CORES=8):
        self.N, self.E, self.D = N, E, D
        self.DEG, self.DEP, self.K, self.CORES = DEG, DEP, K, CORES
        self.NS = N // CORES
        self.ES = E // CORES
        self.NB = self.NS // P
        self.EB = self.ES // P
        self.DC = D // P
        self.KCN = (2 * D) // P
        self.KCE = (3 * D) // P
        assert self.NS % P == 0 and self.ES % P == 0 and D % P == 0
        assert 2 * N <= 32768 and E <= 32768  # int16 dma_gather indices


def _ceil128(x):
    return -(-x // 128)


def _pieces(n):
    out = []
    off = 0
    while off < n:
        out.append(min(CMAX, n - off))
        off += CMAX
    return out


class Plan:
    """Host-derived compile-time structure (chunk counts, column offsets),
    maxed across cores so one SPMD program fits all cores."""

    def __init__(self, cfg, inputs):
        NS, ES, NB, EB = cfg.NS, cfg.ES, cfg.NB, cfg.EB
        C = cfg.CORES
        adj = {0: np.asarray(inputs["fw_adj"], np.int64),
               1: np.asarray(inputs["bw_adj"], np.int64)}
        eid = {0: np.asarray(inputs["fw_edgeid"], np.int64),
               1: np.asarray(inputs["bw_edgeid"], np.int64)}
        dep = {0: np.asarray(inputs["fw_edgedep"], np.int64),
               1: np.asarray(inputs["bw_edgedep"], np.int64)}

        ES_LO = (ES * 5) // 8

        def e_half(v):
            # edge id -> which half-table it lives in (-1 for padding)
            return np.where(v < 0, -1, ((v % ES) >= ES_LO).astype(np.int64))

        self.Ka = np.zeros((2, NB), np.int64)
        self.Ke = np.zeros((2, 2, NB), np.int64)  # [half, d, b]
        self.Kf = np.zeros(EB, np.int64)
        self.Kb = np.zeros(EB, np.int64)
        for d in (0, 1):
            for b in range(NB):
                for c in range(C):
                    r0 = c * NS + b * P
                    self.Ka[d, b] = max(self.Ka[d, b],
                                        _ceil128((adj[d][r0:r0 + P] >= 0).sum()))
                    eh = e_half(eid[d][r0:r0 + P])
                    for h in (0, 1):
                        self.Ke[h, d, b] = max(self.Ke[h, d, b],
                                               _ceil128((eh == h).sum()))
        for b in range(EB):
            for c in range(C):
                r0 = c * ES + b * P
                self.Kf[b] = max(self.Kf[b], _ceil128((dep[0][r0:r0 + P] >= 0).sum()))
                self.Kb[b] = max(self.Kb[b], _ceil128((dep[1][r0:r0 + P] >= 0).sum()))

        self.n_off = np.zeros((2, NB), np.int64)
        off = 0
        for d in (0, 1):
            for b in range(NB):
                self.n_off[d, b] = off
                off += self.Ka[d, b] + self.Ke[0, d, b] + self.Ke[1, d, b]
        self.n_chunks = off
        self.e_off = np.zeros(EB, np.int64)
        off = 0
        for b in range(EB):
            self.e_off[b] = off
            off += self.Kf[b] + self.Kb[b]
        self.e_chunks = off

        # Per-piece transfer counts: max over cores of the piece's valid
        # count (compile-time constants; shorter cores pad with fake idx-0 /
        # dest=-1 entries up to the max, -1 skip-tail beyond). Keyed by
        # (kind, d_or_none, b, piece_idx) in issue order per block.
        def counts(vals_by_core, kch):
            per_core = [int((v >= 0).sum()) for v in vals_by_core]
            cnts = []
            off = 0
            for nch in _pieces(kch):
                c = max(min(max(vc - off * 128, 0), nch * 128)
                        for vc in per_core)
                cnts.append(max(c, 1))
                off += nch
            return cnts

        self.cnt_a = {}
        self.cnt_e = {}
        for d in (0, 1):
            for b in range(NB):
                rows = [adj[d][c * NS + b * P: c * NS + (b + 1) * P]
                        for c in range(C)]
                self.cnt_a[d, b] = counts(rows, int(self.Ka[d, b]))
                for h in (0, 1):
                    rows = [np.where(
                        e_half(eid[d][c * NS + b * P: c * NS + (b + 1) * P])
                        == h, 0, -1) for c in range(C)]
                    self.cnt_e[h, d, b] = counts(rows, int(self.Ke[h, d, b]))
        self.cnt_f = {}
        self.cnt_b = {}
        for b in range(EB):
            rows = [dep[0][c * ES + b * P: c * ES + (b + 1) * P]
                    for c in range(C)]
            self.cnt_f[b] = counts(rows, int(self.Kf[b]))
            rows = [dep[1][c * ES + b * P: c * ES + (b + 1) * P]
                    for c in range(C)]
            self.cnt_b[b] = counts(rows, int(self.Kb[b]))

        self.sig = (tuple(self.Ka.ravel()), tuple(self.Ke.ravel()),
                    tuple(self.Kf), tuple(self.Kb),
                    tuple(tuple(v) for v in self.cnt_a.values()),
                    tuple(tuple(v) for v in self.cnt_e.values()),
                    tuple(tuple(v) for v in self.cnt_f.values()),
                    tuple(tuple(v) for v in self.cnt_b.values()))


def build(cfg: Cfg, plan: Plan, pf=4):
    N, E, D = cfg.N, cfg.E, cfg.D
    K, CORES = cfg.K, cfg.CORES
    NS, ES, NB, EB = cfg.NS, cfg.ES, cfg.NB, cfg.EB
    DC, KCN, KCE = cfg.DC, cfg.KCN, cfg.KCE
    Ka, Ke, Kf, Kb = plan.Ka, plan.Ke, plan.Kf, plan.Kb
    KA_MAX = min(int(Ka.max()), CMAX)
    KE_MAX = min(int(Ke.max()), CMAX)
    KD_MAX = min(int(max(Kf.max(), Kb.max())), CMAX)
    SELA_MAX = int(Ka.max())
    SELE_MAX = int((Ke[0] + Ke[1]).max())
    SELD_MAX = int((Kf + Kb).max())
    ES_LO = (cfg.ES * 5) // 8
    ES_HI = cfg.ES - ES_LO
    E_LO = CORES * ES_LO
    E_HI = CORES * ES_HI

    nc = Bacc("TRN2", target_bir_lowering=False, debug=False, num_devices=CORES,
              num_swdge_queues=4)

    # ---- external inputs ----
    fw_tab0 = nc.dram_tensor("fw_tab0", [N, D], FP8, kind="ExternalInput")
    bw_tab0 = nc.dram_tensor("bw_tab0", [N, D], FP8, kind="ExternalInput")
    e_tab0_lo = nc.dram_tensor("e_tab0_lo", [E_LO, D], FP8, kind="ExternalInput")
    e_tab0_hi = nc.dram_tensor("e_tab0_hi", [E_HI, D], FP8, kind="ExternalInput")
    fw_own0 = nc.dram_tensor("fw_own0", [NS, D], BF16, kind="ExternalInput")
    bw_own0 = nc.dram_tensor("bw_own0", [NS, D], BF16, kind="ExternalInput")
    e_own0 = nc.dram_tensor("e_own0", [ES, D], BF16, kind="ExternalInput")
    idx_n = nc.dram_tensor("idx_n", [P, plan.n_chunks * 8], I16, kind="ExternalInput")
    idx_e = nc.dram_tensor("idx_e", [P, plan.e_chunks * 8], I16, kind="ExternalInput")
    dest_n = nc.dram_tensor("dest_n", [P, plan.n_chunks], BF16, kind="ExternalInput")
    dest_e = nc.dram_tensor("dest_e", [P, plan.e_chunks], BF16, kind="ExternalInput")
    rcn_x = nc.dram_tensor("rcn", [P, 2 * NB], F32, kind="ExternalInput")
    rcef_x = nc.dram_tensor("rcef", [P, EB], F32, kind="ExternalInput")
    rceb_x = nc.dram_tensor("rceb", [P, EB], F32, kind="ExternalInput")
    wfc_x = nc.dram_tensor("wfc", [P, KCN * D], BF16, kind="ExternalInput")
    wbc_x = nc.dram_tensor("wbc", [P, KCN * D], BF16, kind="ExternalInput")
    wed_x = nc.dram_tensor("wed", [P, KCE * D], BF16, kind="ExternalInput")
    bfc_x = nc.dram_tensor("bfc", [1, D], BF16, kind="ExternalInput")
    bbc_x = nc.dram_tensor("bbc", [1, D], BF16, kind="ExternalInput")
    bed_x = nc.dram_tensor("bed", [1, D], BF16, kind="ExternalInput")
    fw_out = nc.dram_tensor("fw_out", [NS, D], F32, kind="ExternalOutput")
    bw_out = nc.dram_tensor("bw_out", [NS, D], F32, kind="ExternalOutput")

    rg = [list(range(CORES))]
    RELU = mybir.ActivationFunctionType.Relu
    COPY = mybir.ActivationFunctionType.Copy
    EQ = mybir.AluOpType.is_equal

    with tile.TileContext(nc) as tc:
        with (
            tc.tile_pool(name="const", bufs=1) as cp,
            tc.tile_pool(name="gp", bufs=2) as gp,
            tc.tile_pool(name="slp", bufs=3) as slp,
            tc.tile_pool(name="sp", bufs=3) as sp,
            tc.tile_pool(name="xp", bufs=2) as xp,
            tc.tile_pool(name="op", bufs=3) as op,
            tc.tile_pool(name="pm", bufs=2, space="PSUM") as pmp,
            tc.tile_pool(name="pt", bufs=1, space="PSUM") as ptp,
            tc.tile_pool(name="po", bufs=2, space="PSUM") as pop,
            tc.tile_pool(name="dram", bufs=1, space="DRAM") as dp,
        ):
            # ---- constants ----
            ident = cp.tile([P, P], BF16)
            make_identity(nc, ident[:])
            ones1 = cp.tile([1, P], BF16)
            nc.gpsimd.memset(ones1[:], 1.0)
            iota_i = cp.tile([P, P], I32)
            nc.gpsimd.iota(iota_i[:], pattern=[[1, P]], base=0,
                           channel_multiplier=0)
            iota_b = cp.tile([P, P], BF16)
            nc.vector.tensor_copy(out=iota_b[:], in_=iota_i[:])

            def load_flat(name, src, shape, dt):
                t = cp.tile(shape, dt, name=name)
                nc.sync.dma_start(out=t[:], in_=src[:])
                return t

            ixn_t = load_flat("ixn_t", idx_n, [P, plan.n_chunks * 8], I16)
            dn_t = load_flat("dn_t", dest_n, [P, plan.n_chunks], BF16)
            rcn_t = load_flat("rcn_t", rcn_x, [P, 2 * NB], F32)
            ixe_t = load_flat("ixe_t", idx_e, [P, plan.e_chunks * 8], I16)
            de_t = load_flat("de_t", dest_e, [P, plan.e_chunks], BF16)
            wfc_t = load_flat("wfc_t", wfc_x, [P, KCN * D], BF16)
            wbc_t = load_flat("wbc_t", wbc_x, [P, KCN * D], BF16)
            wed_t = load_flat("wed_t", wed_x, [P, KCE * D], BF16)
            bfc_t = load_flat("bfc_t", bfc_x, [1, D], BF16)
            bbc_t = load_flat("bbc_t", bbc_x, [1, D], BF16)
            bed_t = load_flat("bed_t", bed_x, [1, D], BF16)
            rcef_t = load_flat("rcef_t", rcef_x, [P, EB], F32)
            rceb_t = load_flat("rceb_t", rceb_x, [P, EB], F32)

            # ---- DRAM tables / staging ----
            fw_tabs = [fw_tab0] + [dp.tile([N, D], BF16, addr_space="Shared",
                                           name=f"fw_tab{k}") for k in (1, 2)]
            bw_tabs = [bw_tab0] + [dp.tile([N, D], BF16, addr_space="Shared",
                                           name=f"bw_tab{k}") for k in (1, 2)]
            n_tabs = [(fw_tabs[k], bw_tabs[k]) for k in range(K)]
            # per-direction fp8 node tables (edge-phase dep gathers): the
            # bw AllGather triggers right after the bw node blocks and hides
            # under the fw node phase, so edge gb gathers start immediately.
            fw_tabq = [None] + [dp.tile([N, D], FP8, addr_space="Shared",
                                        name=f"fw_tq{k}") for k in (1, 2)]
            bw_tabq = [None] + [dp.tile([N, D], FP8, addr_space="Shared",
                                        name=f"bw_tq{k}") for k in (1, 2)]
            e_tabs = [(e_tab0_lo, e_tab0_hi)] + [
                (dp.tile([E_LO, D], FP8, addr_space="Shared", name=f"e_tl{k}"),
                 dp.tile([E_HI, D], FP8, addr_space="Shared", name=f"e_th{k}"))
                for k in (1, 2)]
            fw_st = [fw_own0, dp.tile([NS, D], BF16, name="fw_shA"),
                     dp.tile([NS, D], BF16, name="fw_shB")]
            bw_st = [bw_own0, dp.tile([NS, D], BF16, name="bw_shA"),
                     dp.tile([NS, D], BF16, name="bw_shB")]
            fw_stq = [None, dp.tile([NS, D], FP8, name="fw_qA"),
                      dp.tile([NS, D], FP8, name="fw_qB")]
            bw_stq = [None, dp.tile([NS, D], FP8, name="bw_qA"),
                      dp.tile([NS, D], FP8, name="bw_qB")]
            e_st = [e_own0, dp.tile([ES, D], BF16, name="e_shA"),
                    dp.tile([ES, D], BF16, name="e_shB")]
            e_stq = [None, dp.tile([ES, D], FP8, name="e_qA"),
                     dp.tile([ES, D], FP8, name="e_qB")]

            qctr = [0]

            def gather(tab_ap, idx_tile, chunk_off, nchunks, tag, maxch, dt,
                       cnts, bufs=None, into=None, into_col=0,
                       full_cnt=False):
                """ceil(nchunks/CMAX) dma_gather calls -> [(tile, col, nch)].
                cnts[i] = compile-time transfer count (max across cores).
                into/into_col: write into an existing tile at a chunk col."""
                out = []
                off = 0
                pi = 0
                while off < nchunks:
                    nch = min(CMAX, nchunks - off)
                    if into is None:
                        g = gp.tile([P, min(maxch, CMAX) * D], dt,
                                    name=f"g_{tag}", tag=tag, bufs=bufs)
                        col = 0
                    else:
                        g = into
                        col = into_col + off
                    qctr[0] = (qctr[0] + 1) % 4
                    nc.gpsimd.dma_gather(
                        out_ap=g[:, col * D:(col + nch) * D]
                            .rearrange("p (t e) -> p t e", e=D),
                        in_ap=tab_ap,
                        idxs_ap=idx_tile[:, (chunk_off + off) * 8:
                                         (chunk_off + off + nch) * 8],
                        num_idxs=nch * P,
                        num_idxs_reg=nch * P if full_cnt else int(cnts[pi]),
                        elem_size=D,
                        queue_num=qctr[0],
                    )
                    out.append((g, col, nch))
                    off += nch
                    pi += 1
                return out

            def allgather(src_ap, dst_ap):
                nc.gpsimd.collective_compute(
                    "AllGather", mybir.AluOpType.bypass, replica_groups=rg,
                    ins=[src_ap], outs=[dst_ap],
                )

            def ag_rows(st, tab, rows_total, r0, r1):
                """AllGather staging rows [r0:r1) into the strided full-table
                view [C, rows_total, D][:, r0:r1, :]."""
                dst = tab[:].rearrange("(c r) d -> c r d", r=rows_total)
                allgather(st[r0:r1, :], dst[:, r0:r1, :])

            def build_sel(dtile, co, nch, dt, tag, smax):
                """[128, nch*128] selector: sel[r, c*128+p] =
                (dest[r, co+c] == p)."""
                st = slp.tile([P, smax * P], dt, name=f"sel_{tag}", tag=tag)
                io_b = iota_b[:].rearrange("p (o f) -> p o f", o=1) \
                                .broadcast_to([P, nch, P])
                db = dtile[:, co:co + nch].rearrange("p (c o) -> p c o", o=1) \
                                          .broadcast_to([P, nch, P])
                nc.vector.tensor_tensor(
                    out=st[:, :nch * P].rearrange("p (c f) -> p c f", f=P),
                    in0=io_b, in1=db, op=EQ)
                return st

            def flat_chunks(glist):
                return [(g, col + c) for g, col, n in glist for c in range(n)]

            DR = mybir.MatmulPerfMode.DoubleRow

            def sel_matmul(ps, sel_t, c0, chunks, first, last):
                # pair adjacent fp8 chunks from the same gather tile into
                # DoubleRow matmuls (2 fp8 weights per PE cell)
                groups = []
                i = 0
                while i < len(chunks):
                    g, c = chunks[i]
                    if (sel_t.dtype == FP8 and i + 1 < len(chunks)
                            and chunks[i + 1][0] is g
                            and chunks[i + 1][1] == c + 1):
                        groups.append((g, c, i, True))
                        i += 2
                    else:
                        groups.append((g, c, i, False))
                        i += 1
                for gi, (g, c, i, dbl) in enumerate(groups):
                    st = first and gi == 0
                    sp_ = last and gi == len(groups) - 1
                    if dbl:
                        nc.tensor.matmul(
                            out=ps,
                            lhsT=sel_t[:, (c0 + i) * P:(c0 + i + 2) * P]
                                .rearrange("p (k m) -> p k m", k=2),
                            rhs=g[:, c * D:(c + 2) * D]
                                .rearrange("p (k d) -> p k d", k=2),
                            start=st, stop=sp_, perf_mode=DR,
                        )
                    else:
                        nc.tensor.matmul(
                            out=ps,
                            lhsT=sel_t[:, (c0 + i) * P:(c0 + i + 1) * P],
                            rhs=g[:, c * D:(c + 1) * D],
                            start=st, stop=sp_,
                        )

            def transpose_into(pt, cbase, src_sb, nch):
                for c in range(nch):
                    nc.tensor.transpose(
                        out=pt[:, (cbase + c) * P:(cbase + c + 1) * P],
                        in_=src_sb[:, c * P:(c + 1) * P],
                        identity=ident[:],
                    )

            def linear(xT, kc, w_t, b_row):
                ps = pop.tile([P, D], F32, name="ps_o", tag="ps_o")
                for kk in range(kc):
                    nc.tensor.matmul(
                        out=ps[:], lhsT=xT[:, kk * P:(kk + 1) * P],
                        rhs=w_t[:, kk * D:(kk + 1) * D],
                        start=(kk == 0), stop=False,
                    )
                nc.tensor.matmul(out=ps[:], lhsT=ones1[:], rhs=b_row[:],
                                 start=False, stop=True)
                return ps

            GA_BUFS = 2 * (pf + 1)

            # Zero every gather-ring buffer once: skipped -1 tails leave
            # stale SBUF that the selector matmuls read (zero-selector), and
            # uninitialized bits could decode as NaN (0 * NaN = NaN).
            for i in range(GA_BUFS):
                t = gp.tile([P, min(KA_MAX, CMAX) * D], BF16, name="z_ga",
                            tag="ga", bufs=GA_BUFS)
                eng = nc.vector if i % 2 == 0 else nc.gpsimd
                eng.memset(t[:], 0.0)
            for i in range(4):
                t = gp.tile([P, min(KE_MAX, CMAX) * D], FP8, name="z_ge",
                            tag="ge", bufs=4)
                eng = nc.gpsimd if i % 2 == 0 else nc.vector
                eng.memset(t[:], 0.0)

            def node_adj_gather(k, d, b):
                tab = n_tabs[k][d]
                dt = FP8 if k == 0 else BF16
                return gather(tab[:], ixn_t, int(plan.n_off[d, b]),
                              int(Ka[d, b]), "ga", KA_MAX, dt,
                              plan.cnt_a[d, b], bufs=GA_BUFS)

            def node_ge_lo(k, d, b):
                ke0 = int(Ke[0, d, b])
                co = int(plan.n_off[d, b]) + int(Ka[d, b])
                lo = gather(e_tabs[k][0][:], ixn_t, co, ke0, "ge", KE_MAX,
                            FP8, plan.cnt_e[0, d, b], bufs=4)
                return True, lo

            def node_ge_hi(k, d, b, gt):
                ke0, ke1 = int(Ke[0, d, b]), int(Ke[1, d, b])
                co = int(plan.n_off[d, b]) + int(Ka[d, b])
                return gather(e_tabs[k][1][:], ixn_t, co + ke0, ke1, "ge",
                              KE_MAX, FP8, plan.cnt_e[1, d, b], bufs=4)

            def node_block(k, d, b, ga, ge):
                last = (k == K - 1)
                ka = int(Ka[d, b])
                ke0, ke1 = int(Ke[0, d, b]), int(Ke[1, d, b])
                ke = ke0 + ke1
                co = int(plan.n_off[d, b])
                adt = FP8 if k == 0 else BF16
                sel_a = build_sel(dn_t, co, ka, adt, "sela", SELA_MAX)
                sel_e = build_sel(dn_t, co + ka, ke, FP8, "sele", SELE_MAX)
                own = sp.tile([P, D], BF16, name="own", tag="own")
                st = fw_st[k] if d == 0 else bw_st[k]
                nc.sync.dma_start(out=own[:], in_=st[b * P:(b + 1) * P, :])

                ps_m = pmp.tile([P, D], F32, name="ps_m", tag="ps_f")
                sel_matmul(ps_m[:], sel_a, 0, flat_chunks(ga),
                           True, ke == 0)
                sel_matmul(ps_m[:], sel_e, 0, flat_chunks(ge),
                           ka == 0, True)
                m_sb = sp.tile([P, D], BF16, name="m_sb", tag="m")
                nc.scalar.activation(out=m_sb[:], in_=ps_m[:], func=COPY,
                                     scale=rcn_t[:, d * NB + b:d * NB + b + 1])

                pt = ptp.tile([P, KCN * P], BF16, name="pt", tag="pt")
                transpose_into(pt, 0, own[:], DC)
                transpose_into(pt, DC, m_sb[:], DC)
                xT = xp.tile([P, KCN * P], BF16, name="xT", tag="xT")
                nc.vector.tensor_copy(out=xT[:], in_=pt[:])

                w_t = wfc_t if d == 0 else wbc_t
                b_row = bfc_t if d == 0 else bbc_t
                ps_o = linear(xT, KCN, w_t, b_row)
                if not last:
                    ob = op.tile([P, D], BF16, name="ob", tag="ob")
                    nc.scalar.activation(out=ob[:], in_=ps_o[:], func=RELU)
                    obq = op.tile([P, D], FP8, name="obq", tag="obq")
                    nc.scalar.activation(out=obq[:], in_=ps_o[:], func=RELU)
                    dst = fw_st[k + 1] if d == 0 else bw_st[k + 1]
                    dstq = fw_stq[k + 1] if d == 0 else bw_stq[k + 1]
                    nc.sync.dma_start(out=dst[b * P:(b + 1) * P, :], in_=ob[:])
                    nc.sync.dma_start(out=dstq[b * P:(b + 1) * P, :],
                                      in_=obq[:])
                else:
                    of = op.tile([P, D], F32, name="of", tag="of")
                    nc.scalar.activation(out=of[:], in_=ps_o[:], func=COPY)
                    dst = fw_out if d == 0 else bw_out
                    nc.sync.dma_start(out=dst[b * P:(b + 1) * P, :], in_=of[:])

            def edge_gb_gather(k, b):
                kf, kb = int(Kf[b]), int(Kb[b])
                co = int(plan.e_off[b])
                return gather(bw_tabq[k + 1][:], ixe_t, co + kf, kb, "gd",
                              KD_MAX, FP8, plan.cnt_b[b], bufs=8)

            def edge_gf_gather(k, b):
                kf = int(Kf[b])
                co = int(plan.e_off[b])
                return gather(fw_tabq[k + 1][:], ixe_t, co, kf, "gd",
                              KD_MAX, FP8, plan.cnt_f[b], bufs=8)

            def edge_block(k, b, gb, gf):
                kf, kb = int(Kf[b]), int(Kb[b])
                co = int(plan.e_off[b])
                sel_t = build_sel(de_t, co, kf + kb, FP8, "seld", SELD_MAX)
                own = sp.tile([P, D], BF16, name="own_e", tag="own")
                nc.sync.dma_start(out=own[:],
                                  in_=e_st[k][b * P:(b + 1) * P, :])

                # bw half first, fw half second (independent PSUM tiles so
                # each half retires on its own).
                ps_b = pmp.tile([P, D], F32, name="ps_be", tag="ps_b")
                sel_matmul(ps_b[:], sel_t, kf, flat_chunks(gb),
                           True, True)
                mb = sp.tile([P, D], BF16, name="mb", tag="m2")
                nc.scalar.activation(out=mb[:], in_=ps_b[:], func=COPY,
                                     scale=rceb_t[:, b:b + 1])

                ps_f = pmp.tile([P, D], F32, name="ps_fe", tag="ps_f")
                sel_matmul(ps_f[:], sel_t, 0, flat_chunks(gf),
                           True, True)
                mf = sp.tile([P, D], BF16, name="mf", tag="m")
                nc.scalar.activation(out=mf[:], in_=ps_f[:], func=COPY,
                                     scale=rcef_t[:, b:b + 1])

                pt = ptp.tile([P, KCE * P], BF16, name="pt_e", tag="pt")
                transpose_into(pt, 0, own[:], DC)
                transpose_into(pt, DC, mf[:], DC)
                transpose_into(pt, 2 * DC, mb[:], DC)
                xT = xp.tile([P, KCE * P], BF16, name="xT_e", tag="xT")
                nc.vector.tensor_copy(out=xT[:], in_=pt[:])

                ps_o = linear(xT, KCE, wed_t, bed_t)
                eb = op.tile([P, D], BF16, name="eb", tag="ob")
                nc.scalar.activation(out=eb[:], in_=ps_o[:], func=RELU)
                ebq = op.tile([P, D], FP8, name="ebq", tag="obq")
                nc.scalar.activation(out=ebq[:], in_=ps_o[:], func=RELU)
                nc.sync.dma_start(out=e_st[k + 1][b * P:(b + 1) * P, :],
                                  in_=eb[:])
                nc.sync.dma_start(out=e_stq[k + 1][b * P:(b + 1) * P, :],
                                  in_=ebq[:])

            # ---------------- program ----------------
            def prefetch_unit(k, d, b, with_lo):
                ent = {"ga": node_adj_gather(k, d, b), "gt": None, "lo": None}
                if with_lo:
                    ent["gt"], ent["lo"] = node_ge_lo(k, d, b)
                return ent

            units = [(d, b) for d in (1, 0) for b in range(NB)]
            pend = [prefetch_unit(0, *units[j], with_lo=(j < 2))
                    for j in range(pf)]
            EPF = 6
            FPF = 3
            for k in range(K):
                epend = None
                for ui, (d, b) in enumerate(units):
                    if k == 0 and ui == 0:
                        # edge-gather ring is first touched in the edge
                        # phase: zero it during the node phase
                        for i in range(8):
                            t = gp.tile([P, min(KD_MAX, CMAX) * D], FP8,
                                        name="z_gd", tag="gd", bufs=8)
                            nc.vector.memset(t[:], 0.0)
                    ent = pend[ui]
                    if ui + pf < len(units):
                        pend.append(
                            prefetch_unit(k, *units[ui + pf], with_lo=False))
                    if ent["gt"] is None:
                        ent["gt"], ent["lo"] = node_ge_lo(k, d, b)
                    ge = ent["lo"] + node_ge_hi(k, d, b, ent["gt"])
                    node_block(k, d, b, ent["ga"], ge)
                    if k < K - 1 and b == NB - 1:
                        # per-direction fp8 AllGather right behind its last
                        # producing block (bw first, so its AG hides under
                        # the fw node phase)
                        stq = fw_stq[k + 1] if d == 0 else bw_stq[k + 1]
                        tabq = fw_tabq[k + 1] if d == 0 else bw_tabq[k + 1]
                        allgather(stq[:], tabq[:])
                    if k < K - 1 and ui == len(units) - 3:
                        # edge-phase bw-dep gathers depend only on the bwq
                        # AllGather (done mid-fw-phase): issue them before
                        # the last fw blocks so their data is resident when
                        # the edge phase starts.
                        epend = [edge_gb_gather(k, j) for j in range(EPF)]
                if k < K - 1:
                    pend = []
                    fpend = [edge_gf_gather(k, b) for b in range(FPF)]
                    for b in range(EB):
                        if b + EPF < EB:
                            epend.append(edge_gb_gather(k, b + EPF))
                        if b + FPF < EB:
                            fpend.append(edge_gf_gather(k, b + FPF))
                        edge_block(k, b, epend[b], fpend[b])
                        # bf16 node tables are only needed by hop k+1's
                        # adjacency gathers: all-gather them during the edge
                        # phase, behind the critical fp8 AllGathers.
                        if b == 0:
                            allgather(bw_st[k + 1][:], bw_tabs[k + 1][:])
                        elif b == 1:
                            allgather(fw_st[k + 1][:], fw_tabs[k + 1][:])
                        elif b == ES_LO // P - 1:
                            # lo part (5/8) of the edge table: AllGather
                            # overlaps the remaining edge blocks; the exposed
                            # hi AllGather at the hop boundary shrinks.
                            dst = e_tabs[k + 1][0][:].rearrange(
                                "(c r) d -> c r d", r=ES_LO)
                            allgather(e_stq[k + 1][0:ES_LO, :], dst)
                    pend = [prefetch_unit(k + 1, *units[j], with_lo=(j < 2))
                            for j in range(pf)]
                    dst = e_tabs[k + 1][1][:].rearrange("(c r) d -> c r d",
                                                        r=ES_HI)
                    allgather(e_stq[k + 1][ES_LO:ES, :], dst)

    # Rebind SWDGE queue_num to the scheduled DMASW lane so each completion
    # semaphore always fires from one queue (the tile scheduler reorders
    # Pool DMA instructions).
    from concourse.tile_sem_assignment import PROC_NAME_TO_IDX
    idx_to_proc = {v: k for k, v in PROC_NAME_TO_IDX.items()}
    for blk in nc.m.functions[0].blocks:
        for inst in blk.instructions:
            if (inst.engine == mybir.EngineType.Pool
                    and hasattr(inst, "queue_num")
                    and getattr(inst, "bass_scheduled_proc", None) is not None):
                pname = idx_to_proc.get(inst.bass_scheduled_proc, "")
                if isinstance(pname, str) and pname.startswith("DMASW"):
                    inst.queue_num = int(pname[5:]) % 4

    nc.compile()
    return nc


def _pack_idx(lst):
    """[m] int (m % 128 == 0) -> [128, m/16] int16 wrapped gather layout."""
    wrapped = lst.astype(np.int16).reshape(-1, 16).T
    return np.tile(wrapped, (8, 1))


def prep_inputs(cfg: Cfg, plan: Plan, inputs: dict):
    import ml_dtypes
    bf16 = ml_dtypes.bfloat16
    fp8 = ml_dtypes.float8_e4m3
    N, E, D = cfg.N, cfg.E, cfg.D
    NS, ES, NB, EB, C = cfg.NS, cfg.ES, cfg.NB, cfg.EB, cfg.CORES
    KCN, KCE = cfg.KCN, cfg.KCE
    f32 = np.float32

    fw = np.asarray(inputs["fw_input"], f32)
    bw = np.asarray(inputs["bw_input"], f32)
    ee = np.asarray(inputs["edge_embs"], f32)
    adj = {0: np.asarray(inputs["fw_adj"], np.int64),
           1: np.asarray(inputs["bw_adj"], np.int64)}
    eid = {0: np.asarray(inputs["fw_edgeid"], np.int64),
           1: np.asarray(inputs["bw_edgeid"], np.int64)}
    dep = {0: np.asarray(inputs["fw_edgedep"], np.int64),
           1: np.asarray(inputs["bw_edgedep"], np.int64)}

    def wchunks(W, kc):
        W = np.asarray(W, f32)
        return np.concatenate([W[kk * P:(kk + 1) * P, :] for kk in range(kc)],
                              axis=1).astype(bf16)

    wfc = wchunks(inputs["Wfc"], KCN)
    wbc = wchunks(inputs["Wbc"], KCN)
    wed = wchunks(inputs["Wedge"], KCE)
    bfc = np.asarray(inputs["bfc"], f32).reshape(1, D).astype(bf16)
    bbc = np.asarray(inputs["bbc"], f32).reshape(1, D).astype(bf16)
    bed = np.asarray(inputs["bedge"], f32).reshape(1, D).astype(bf16)

    fw_tab0 = fw.astype(fp8)
    bw_tab0 = bw.astype(fp8)
    ES_LO = (ES * 5) // 8
    ES_HI = ES - ES_LO
    ee_r = ee.reshape(C, ES, D)
    e_tab0_lo = ee_r[:, :ES_LO].reshape(C * ES_LO, D).astype(fp8)
    e_tab0_hi = ee_r[:, ES_LO:].reshape(C * ES_HI, D).astype(fp8)

    def e_remap(v):
        # global edge id -> (half, row within half-table)
        cown = v // ES
        j = v % ES
        h = (j >= ES_LO).astype(np.int64)
        return h, np.where(h == 0, cown * ES_LO + j,
                           cown * ES_HI + (j - ES_LO))

    def pad_lists(vals, msk, kch, cnts):
        """valid list -> [kch*128]: valid entries, fake idx-0 fill up to
        each piece's shared count, -1 skip-tail beyond."""
        lst = vals[msk]
        m = kch * P
        lpad = np.full(m, -1, np.int64)
        lpad[:len(lst)] = lst
        off = 0
        for nch, cnt in zip(_pieces(kch), cnts):
            have = min(max(len(lst) - off * P, 0), nch * P)
            lpad[off * P + have: off * P + cnt] = 0
            off += nch
        return lpad

    in_maps = []
    for c in range(C):
        idx_cols = []
        dest_n = np.full((P, plan.n_chunks), -1.0, f32)
        rcn = np.zeros((P, 2 * NB), f32)
        for d in (0, 1):
            for b in range(NB):
                r0 = c * NS + b * P
                ka = int(plan.Ka[d, b])
                co = int(plan.n_off[d, b])
                av = adj[d][r0:r0 + P]
                ev = eid[d][r0:r0 + P]
                am, em = av >= 0, ev >= 0
                rcn[:, d * NB + b] = 1.0 / (am.sum(1) + em.sum(1))
                eh, erow = e_remap(np.maximum(ev, 0))
                eh = np.where(em, eh, -1)
                ke0 = int(plan.Ke[0, d, b])
                for (vals, msk, kch, base, cnts) in (
                        (av, am, ka, co, plan.cnt_a[d, b]),
                        (erow, eh == 0, ke0, co + ka, plan.cnt_e[0, d, b]),
                        (erow, eh == 1, int(plan.Ke[1, d, b]), co + ka + ke0,
                         plan.cnt_e[1, d, b])):
                    pidx, _ = np.nonzero(msk)
                    lst = vals[msk]
                    lpad = pad_lists(vals, msk, kch, cnts)
                    idx_cols.append(_pack_idx(lpad))
                    i = np.arange(len(lst))
                    dest_n[i % P, base + i // P] = pidx
        idx_n = np.concatenate(idx_cols, axis=1)

        idx_cols = []
        dest_e = np.full((P, plan.e_chunks), -1.0, f32)
        rcef = np.zeros((P, EB), f32)
        rceb = np.zeros((P, EB), f32)
        for b in range(EB):
            r0 = c * ES + b * P
            kf, kb = int(plan.Kf[b]), int(plan.Kb[b])
            co = int(plan.e_off[b])
            fv, bv = dep[0][r0:r0 + P], dep[1][r0:r0 + P]
            fm, bm = fv >= 0, bv >= 0
            rcef[:, b] = 1.0 / fm.sum(1)
            rceb[:, b] = 1.0 / bm.sum(1)
            for (vals, msk, kch, base, cnts) in (
                    (fv, fm, kf, 0, plan.cnt_f[b]),
                    (bv, bm, kb, kf, plan.cnt_b[b])):
                pidx, _ = np.nonzero(msk)
                lst = vals[msk]
                lpad = pad_lists(vals, msk, kch, cnts)
                idx_cols.append(_pack_idx(lpad))
                i = np.arange(len(lst))
                dest_e[i % P, co + base + i // P] = pidx
        idx_e = np.concatenate(idx_cols, axis=1)

        im = {
            "fw_tab0": fw_tab0, "bw_tab0": bw_tab0,
            "e_tab0_lo": e_tab0_lo, "e_tab0_hi": e_tab0_hi,
            "fw_own0": fw[c * NS:(c + 1) * NS].astype(bf16),
            "bw_own0": bw[c * NS:(c + 1) * NS].astype(bf16),
            "e_own0": ee[c * ES:(c + 1) * ES].astype(bf16),
            "idx_n": idx_n, "idx_e": idx_e,
            "dest_n": dest_n.astype(bf16), "dest_e": dest_e.astype(bf16),
            "rcn": rcn, "rcef": rcef, "rceb": rceb,
            "wfc": wfc, "wbc": wbc, "wed": wed,
            "bfc": bfc, "bbc": bbc, "bed": bed,
        }
        in_maps.append(im)
    return in_maps


def assemble_outputs(cfg: Cfg, results):
    fw = np.concatenate([results[c]["fw_out"] for c in range(cfg.CORES)], axis=0)
    bw = np.concatenate([results[c]["bw_out"] for c in range(cfg.CORES)], axis=0)
    return fw, bw


# ======================= self-contained runner =======================
import os as _os
import types as _types


def _install_axon_prof():
    name = "antenv.axon_hooks"
    if name in sys.modules:
        return True
    try:
        mod = _types.ModuleType(name)
        mod._hook = None
        mod.set_axon_ntff_profile_hook = lambda h: setattr(mod, "_hook", h)
        mod.get_axon_ntff_profile_hook = lambda: mod._hook
        sys.modules[name] = mod
        import antenv
        antenv.axon_hooks = mod
        from trn_agent_boot.trn_boot import _ntff_profile_via_ctypes
        mod.set_axon_ntff_profile_hook(
            _ntff_profile_via_ctypes('/opt/axon/libaxon_pjrt.so'))
        return True
    except Exception:
        sys.modules.pop(name, None)
        return False


_CACHE = {}
LAST_EXEC_NS = None
LAST_PROFILE = None


def kernel(**inputs):
    """Full-input GNN forward on 8 TRN2 NeuronCores. Returns (fw, bw)."""
    global LAST_EXEC_NS, LAST_PROFILE
    from concourse.bass_utils import run_bass_kernel_spmd

    cfg = Cfg()
    plan = Plan(cfg, inputs)
    key = plan.sig
    if _CACHE.get("key") != key:
        _CACHE["nc"] = build(cfg, plan)
        _CACHE["key"] = key
    nc = _CACHE["nc"]

    in_maps = prep_inputs(cfg, plan, inputs)

    profile = _os.environ.get("GNN_PROFILE", "0") == "1"
    if profile:
        profile = _install_axon_prof()
    res = run_bass_kernel_spmd(nc, in_maps, core_ids=list(range(cfg.CORES)),
                               trace=profile)
    LAST_EXEC_NS = res.exec_time_ns
    LAST_PROFILE = res.profile_json
    if res.instructions_and_trace is not None:
        try:
            print("trace:", res.instructions_and_trace[1])
        except Exception:
            pass
    return assemble_outputs(cfg, res.results)


# revision 52
# speedup vs baseline: 1.0581x; 1.0327x over previous
"""GNN message-passing kernel for TRN2, 8-core SPMD (self-contained).

v4 design (on top of v3), ~1.3 ms vs the 2.17 ms v3 baseline:
- Node rows sharded 8 ways (NS=N/8), edge rows too (ES=E/8).
- Mixed-precision gathers: node-adjacency gathers are bf16 at hops 1-2 and
  fp8-e4m3 at hop 0 (the hop-0 table is a host-quantized input, so no AG
  cost); edge-embedding gathers (node phase) and node-dep gathers (edge
  phase) are fp8 everywhere, halving their DMA bytes. CPU-sim rel err of
  this split 1.25e-2, HW 1.27e-2 (< 2e-2 gate).
- fp8 selector matmuls run pairwise in DoubleRow perf mode (2 fp8 weights
  per PE cell): ~2 chunks per 239 ns instead of 2x370 ns.
- Gather counts are compile-time per-piece maxima across cores; shorter
  cores pad with fake idx-0/dest=-1 entries, and the index tail beyond the
  shared count is -1 (SWDGE skips negative tails entirely). Gather-ring
  SBUF is memset once at startup so skipped tails can never feed NaNs into
  the zero-selector matmuls.
- AllGather restructure: small fp8 node tables (needed by the very next
  edge phase) gather right behind the producing node blocks; bf16 node
  tables (needed only by the NEXT hop's adjacency gathers) gather during
  the edge phase, off the critical path. The edge table is split 5/8 : 3/8
  into lo / hi part-tables (separate Shared tensors, host-remapped
  indices): the lo AllGather hides under the remaining edge blocks and the
  smaller hi AllGather shortens the exposed hop-boundary tail; the next
  hop's adjacency + lo-part gathers are prefetched before the hi AllGather
  so they run during it.
- Segment-mean via selector matmuls on the Tensor engine (is_equal-built
  0/1 selectors on the DVE), 1/cnt folded into the PSUM->SBUF activation
  copy. Linear layers bf16, bias via rank-1 matmul, ReLU fused in the
  PSUM->SBUF copy.
"""
import sys

sys.path.insert(0, '/opt/trn_rl_repo')

import numpy as np
import concourse.bass as bass
import concourse.mybir as mybir
from concourse import tile
from concourse.bacc import Bacc
from concourse.masks import make_identity

F32 = mybir.dt.float32
I32 = mybir.dt.int32
BF16 = mybir.dt.bfloat16
FP8 = mybir.dt.float8e4
I16 = mybir.dt.int16
P = 128

CMAX = 8  # max 128-row chunks per dma_gather call (ring capacity)


class Cfg:
    def __init__(self, N=8192, E=32768, D=512, DEG=16, DEP=8, K=3, CORES=8):
        self.N, self.E, self.D = N, E, D
        self.DEG, self.DEP, self.K, self.CORES = DEG, DEP, K, CORES
        self.NS = N // CORES
        self.ES = E // CORES
        self.NB = self.NS // P
        self.EB = self.ES // P
        self.DC = D // P
        self.KCN = (2 * D) // P
        self.KCE = (3 * D) // P
        assert self.NS % P == 0 and self.ES % P == 0 and D % P == 0
        assert 2 * N <= 32768 and E <= 32768  # int16 dma_gather indices


def _ceil128(x):
    return -(-x // 128)


def _pieces(n):
    out = []
    off = 0
    while off < n:
        out.append(min(CMAX, n - off))
        off += CMAX
    return out


class Plan:
    """Host-derived compile-time structure (chunk counts, column offsets),
    maxed across cores so one SPMD program fits all cores."""

    def __init__(self, cfg, inputs):
        NS, ES, NB, EB = cfg.NS, cfg.ES, cfg.NB, cfg.EB
        C = cfg.CORES
        adj = {0: np.asarray(inputs["fw_adj"], np.int64),
               1: np.asarray(inputs["bw_adj"], np.int64)}
        eid = {0: np.asarray(inputs["fw_edgeid"], np.int64),
               1: np.asarray(inputs["bw_edgeid"], np.int64)}
        dep = {0: np.asarray(inputs["fw_edgedep"], np.int64),
               1: np.asarray(inputs["bw_edgedep"], np.int64)}

        ES_LO = (ES * 5) // 8

        def e_half(v):
            # edge id -> which half-table it lives in (-1 for padding)
            return np.where(v < 0, -1, ((v % ES) >= ES_LO).astype(np.int64))

        self.Ka = np.zeros((2, NB), np.int64)
        self.Ke = np.zeros((2, 2, NB), np.int64)  # [half, d, b]
        self.Kf = np.zeros(EB, np.int64)
        self.Kb = np.zeros(EB, np.int64)
        for d in (0, 1):
            for b in range(NB):
                for c in range(C):
                    r0 = c * NS + b * P
                    self.Ka[d, b] = max(self.Ka[d, b],
                                        _ceil128((adj[d][r0:r0 + P] >= 0).sum()))
                    eh = e_half(eid[d][r0:r0 + P])
                    for h in (0, 1):
                        self.Ke[h, d, b] = max(self.Ke[h, d, b],
                                               _ceil128((eh == h).sum()))
        for b in range(EB):
            for c in range(C):
                r0 = c * ES + b * P
                self.Kf[b] = max(self.Kf[b], _ceil128((dep[0][r0:r0 + P] >= 0).sum()))
                self.Kb[b] = max(self.Kb[b], _ceil128((dep[1][r0:r0 + P] >= 0).sum()))

        self.n_off = np.zeros((2, NB), np.int64)
        off = 0
        for d in (0, 1):
            for b in range(NB):
                self.n_off[d, b] = off
                off += self.Ka[d, b] + self.Ke[0, d, b] + self.Ke[1, d, b]
        self.n_chunks = off
        self.e_off = np.zeros(EB, np.int64)
        off = 0
        for b in range(EB):
            self.e_off[b] = off
            off += self.Kf[b] + self.Kb[b]
        self.e_chunks = off

        # Per-piece transfer counts: max over cores of the piece's valid
        # count (compile-time constants; shorter cores pad with fake idx-0 /
        # dest=-1 entries up to the max, -1 skip-tail beyond). Keyed by
        # (kind, d_or_none, b, piece_idx) in issue order per block.
        def counts(vals_by_core, kch):
            per_core = [int((v >= 0).sum()) for v in vals_by_core]
            cnts = []
            off = 0
            for nch in _pieces(kch):
                c = max(min(max(vc - off * 128, 0), nch * 128)
                        for vc in per_core)
                cnts.append(max(c, 1))
                off += nch
            return cnts

        self.cnt_a = {}
        self.cnt_e = {}
        for d in (0, 1):
            for b in range(NB):
                rows = [adj[d][c * NS + b * P: c * NS + (b + 1) * P]
                        for c in range(C)]
                self.cnt_a[d, b] = counts(rows, int(self.Ka[d, b]))
                for h in (0, 1):
                    rows = [np.where(
                        e_half(eid[d][c * NS + b * P: c * NS + (b + 1) * P])
                        == h, 0, -1) for c in range(C)]
                    self.cnt_e[h, d, b] = counts(rows, int(self.Ke[h, d, b]))
        self.cnt_f = {}
        self.cnt_b = {}
        for b in range(EB):
            rows = [dep[0][c * ES + b * P: c * ES + (b + 1) * P]
                    for c in range(C)]
            self.cnt_f[b] = counts(rows, int(self.Kf[b]))
            rows = [dep[1][c * ES + b * P: c * ES + (b + 1) * P]
                    for c in range(C)]
            self.cnt_b[b] = counts(rows, int(self.Kb[b]))

        self.no_bias = all(
            np.all(np.asarray(inputs[k], np.float64) == 0.0)
            for k in ("bfc", "bbc", "bedge"))

        self.sig = (self.no_bias,
                    tuple(self.Ka.ravel()), tuple(self.Ke.ravel()),
                    tuple(self.Kf), tuple(self.Kb),
                    tuple(tuple(v) for v in self.cnt_a.values()),
                    tuple(tuple(v) for v in self.cnt_e.values()),
                    tuple(tuple(v) for v in self.cnt_f.values()),
                    tuple(tuple(v) for v in self.cnt_b.values()))


def build(cfg: Cfg, plan: Plan, pf=4):
    N, E, D = cfg.N, cfg.E, cfg.D
    K, CORES = cfg.K, cfg.CORES
    NS, ES, NB, EB = cfg.NS, cfg.ES, cfg.NB, cfg.EB
    DC, KCN, KCE = cfg.DC, cfg.KCN, cfg.KCE
    Ka, Ke, Kf, Kb = plan.Ka, plan.Ke, plan.Kf, plan.Kb
    KA_MAX = min(int(Ka.max()), CMAX)
    KE_MAX = min(int(Ke.max()), CMAX)
    KD_MAX = min(int(max(Kf.max(), Kb.max())), CMAX)
    SELA_MAX = int(Ka.max())
    SELE_MAX = int((Ke[0] + Ke[1]).max())
    SELD_MAX = int((Kf + Kb).max())
    ES_LO = (cfg.ES * 5) // 8
    ES_HI = cfg.ES - ES_LO
    E_LO = CORES * ES_LO
    E_HI = CORES * ES_HI

    nc = Bacc("TRN2", target_bir_lowering=False, debug=False, num_devices=CORES,
              num_swdge_queues=4)

    # ---- external inputs ----
    fw_tab0 = nc.dram_tensor("fw_tab0", [N, D], FP8, kind="ExternalInput")
    bw_tab0 = nc.dram_tensor("bw_tab0", [N, D], FP8, kind="ExternalInput")
    e_tab0_lo = nc.dram_tensor("e_tab0_lo", [E_LO, D], FP8, kind="ExternalInput")
    e_tab0_hi = nc.dram_tensor("e_tab0_hi", [E_HI, D], FP8, kind="ExternalInput")
    fw_own0 = nc.dram_tensor("fw_own0", [NS, D], BF16, kind="ExternalInput")
    bw_own0 = nc.dram_tensor("bw_own0", [NS, D], BF16, kind="ExternalInput")
    e_own0 = nc.dram_tensor("e_own0", [ES, D], BF16, kind="ExternalInput")
    idx_n = nc.dram_tensor("idx_n", [P, plan.n_chunks * 8], I16, kind="ExternalInput")
    idx_e = nc.dram_tensor("idx_e", [P, plan.e_chunks * 8], I16, kind="ExternalInput")
    dest_n = nc.dram_tensor("dest_n", [P, plan.n_chunks], BF16, kind="ExternalInput")
    dest_e = nc.dram_tensor("dest_e", [P, plan.e_chunks], BF16, kind="ExternalInput")
    rcn_x = nc.dram_tensor("rcn", [P, 2 * NB], F32, kind="ExternalInput")
    rcef_x = nc.dram_tensor("rcef", [P, EB], F32, kind="ExternalInput")
    rceb_x = nc.dram_tensor("rceb", [P, EB], F32, kind="ExternalInput")
    wfc_x = nc.dram_tensor("wfc", [P, KCN * D], BF16, kind="ExternalInput")
    wbc_x = nc.dram_tensor("wbc", [P, KCN * D], BF16, kind="ExternalInput")
    wed_x = nc.dram_tensor("wed", [P, KCE * D], BF16, kind="ExternalInput")
    bfc_x = nc.dram_tensor("bfc", [1, D], BF16, kind="ExternalInput")
    bbc_x = nc.dram_tensor("bbc", [1, D], BF16, kind="ExternalInput")
    bed_x = nc.dram_tensor("bed", [1, D], BF16, kind="ExternalInput")
    fw_out = nc.dram_tensor("fw_out", [NS, D], F32, kind="ExternalOutput")
    bw_out = nc.dram_tensor("bw_out", [NS, D], F32, kind="ExternalOutput")

    rg = [list(range(CORES))]
    RELU = mybir.ActivationFunctionType.Relu
    COPY = mybir.ActivationFunctionType.Copy
    EQ = mybir.AluOpType.is_equal

    with tile.TileContext(nc) as tc:
        with (
            tc.tile_pool(name="const", bufs=1) as cp,
            tc.tile_pool(name="gp", bufs=2) as gp,
            tc.tile_pool(name="slp", bufs=3) as slp,
            tc.tile_pool(name="sp", bufs=3) as sp,
            tc.tile_pool(name="xp", bufs=2) as xp,
            tc.tile_pool(name="op", bufs=3) as op,
            tc.tile_pool(name="pm", bufs=2, space="PSUM") as pmp,
            tc.tile_pool(name="pt", bufs=1, space="PSUM") as ptp,
            tc.tile_pool(name="po", bufs=2, space="PSUM") as pop,
            tc.tile_pool(name="dram", bufs=1, space="DRAM") as dp,
        ):
            # ---- constants ----
            ident = cp.tile([P, P], BF16)
            make_identity(nc, ident[:])
            ones1 = cp.tile([1, P], BF16)
            nc.gpsimd.memset(ones1[:], 1.0)
            iota_i = cp.tile([P, P], I32)
            nc.gpsimd.iota(iota_i[:], pattern=[[1, P]], base=0,
                           channel_multiplier=0)
            iota_b = cp.tile([P, P], BF16)
            nc.vector.tensor_copy(out=iota_b[:], in_=iota_i[:])

            def load_flat(name, src, shape, dt):
                t = cp.tile(shape, dt, name=name)
                nc.sync.dma_start(out=t[:], in_=src[:])
                return t

            ixn_t = load_flat("ixn_t", idx_n, [P, plan.n_chunks * 8], I16)
            dn_t = load_flat("dn_t", dest_n, [P, plan.n_chunks], BF16)
            rcn_t = load_flat("rcn_t", rcn_x, [P, 2 * NB], F32)
            ixe_t = load_flat("ixe_t", idx_e, [P, plan.e_chunks * 8], I16)
            de_t = load_flat("de_t", dest_e, [P, plan.e_chunks], BF16)
            wfc_t = load_flat("wfc_t", wfc_x, [P, KCN * D], BF16)
            wbc_t = load_flat("wbc_t", wbc_x, [P, KCN * D], BF16)
            wed_t = load_flat("wed_t", wed_x, [P, KCE * D], BF16)
            bfc_t = load_flat("bfc_t", bfc_x, [1, D], BF16)
            bbc_t = load_flat("bbc_t", bbc_x, [1, D], BF16)
            bed_t = load_flat("bed_t", bed_x, [1, D], BF16)
            rcef_t = load_flat("rcef_t", rcef_x, [P, EB], F32)
            rceb_t = load_flat("rceb_t", rceb_x, [P, EB], F32)

            # ---- DRAM tables / staging ----
            fw_tabs = [fw_tab0] + [dp.tile([N, D], BF16, addr_space="Shared",
                                           name=f"fw_tab{k}") for k in (1, 2)]
            bw_tabs = [bw_tab0] + [dp.tile([N, D], BF16, addr_space="Shared",
                                           name=f"bw_tab{k}") for k in (1, 2)]
            n_tabs = [(fw_tabs[k], bw_tabs[k]) for k in range(K)]
            # per-direction fp8 node tables (edge-phase dep gathers): the
            # bw AllGather triggers right after the bw node blocks and hides
            # under the fw node phase, so edge gb gathers start immediately.
            fw_tabq = [None] + [dp.tile([N, D], FP8, addr_space="Shared",
                                        name=f"fw_tq{k}") for k in (1, 2)]
            bw_tabq = [None] + [dp.tile([N, D], FP8, addr_space="Shared",
                                        name=f"bw_tq{k}") for k in (1, 2)]
            e_tabs = [(e_tab0_lo, e_tab0_hi)] + [
                (dp.tile([E_LO, D], FP8, addr_space="Shared", name=f"e_tl{k}"),
                 dp.tile([E_HI, D], FP8, addr_space="Shared", name=f"e_th{k}"))
                for k in (1, 2)]
            fw_st = [fw_own0, dp.tile([NS, D], BF16, name="fw_shA"),
                     dp.tile([NS, D], BF16, name="fw_shB")]
            bw_st = [bw_own0, dp.tile([NS, D], BF16, name="bw_shA"),
                     dp.tile([NS, D], BF16, name="bw_shB")]
            fw_stq = [None, dp.tile([NS, D], FP8, name="fw_qA"),
                      dp.tile([NS, D], FP8, name="fw_qB")]
            bw_stq = [None, dp.tile([NS, D], FP8, name="bw_qA"),
                      dp.tile([NS, D], FP8, name="bw_qB")]
            e_st = [e_own0, dp.tile([ES, D], BF16, name="e_shA"),
                    dp.tile([ES, D], BF16, name="e_shB")]
            e_stq = [None, dp.tile([ES, D], FP8, name="e_qA"),
                     dp.tile([ES, D], FP8, name="e_qB")]

            qctr = [0]

            def gather(tab_ap, idx_tile, chunk_off, nchunks, tag, maxch, dt,
                       cnts, bufs=None, into=None, into_col=0,
                       full_cnt=False):
                """ceil(nchunks/CMAX) dma_gather calls -> [(tile, col, nch)].
                cnts[i] = compile-time transfer count (max across cores).
                into/into_col: write into an existing tile at a chunk col."""
                out = []
                off = 0
                pi = 0
                while off < nchunks:
                    nch = min(CMAX, nchunks - off)
                    if into is None:
                        g = gp.tile([P, min(maxch, CMAX) * D], dt,
                                    name=f"g_{tag}", tag=tag, bufs=bufs)
                        col = 0
                    else:
                        g = into
                        col = into_col + off
                    qctr[0] = (qctr[0] + 1) % 4
                    nc.gpsimd.dma_gather(
                        out_ap=g[:, col * D:(col + nch) * D]
                            .rearrange("p (t e) -> p t e", e=D),
                        in_ap=tab_ap,
                        idxs_ap=idx_tile[:, (chunk_off + off) * 8:
                                         (chunk_off + off + nch) * 8],
                        num_idxs=nch * P,
                        num_idxs_reg=nch * P if full_cnt else int(cnts[pi]),
                        elem_size=D,
                        queue_num=qctr[0],
                    )
                    out.append((g, col, nch))
                    off += nch
                    pi += 1
                return out

            def allgather(src_ap, dst_ap):
                nc.gpsimd.collective_compute(
                    "AllGather", mybir.AluOpType.bypass, replica_groups=rg,
                    ins=[src_ap], outs=[dst_ap],
                )

            def ag_rows(st, tab, rows_total, r0, r1):
                """AllGather staging rows [r0:r1) into the strided full-table
                view [C, rows_total, D][:, r0:r1, :]."""
                dst = tab[:].rearrange("(c r) d -> c r d", r=rows_total)
                allgather(st[r0:r1, :], dst[:, r0:r1, :])

            def build_sel(dtile, co, nch, dt, tag, smax):
                """[128, nch*128] selector: sel[r, c*128+p] =
                (dest[r, co+c] == p)."""
                st = slp.tile([P, smax * P], dt, name=f"sel_{tag}", tag=tag)
                io_b = iota_b[:].rearrange("p (o f) -> p o f", o=1) \
                                .broadcast_to([P, nch, P])
                db = dtile[:, co:co + nch].rearrange("p (c o) -> p c o", o=1) \
                                          .broadcast_to([P, nch, P])
                nc.vector.tensor_tensor(
                    out=st[:, :nch * P].rearrange("p (c f) -> p c f", f=P),
                    in0=io_b, in1=db, op=EQ)
                return st

            def flat_chunks(glist):
                return [(g, col + c) for g, col, n in glist for c in range(n)]

            DR = mybir.MatmulPerfMode.DoubleRow

            def sel_matmul(ps, sel_t, c0, chunks, first, last):
                # pair adjacent fp8 chunks from the same gather tile into
                # DoubleRow matmuls (2 fp8 weights per PE cell)
                groups = []
                i = 0
                while i < len(chunks):
                    g, c = chunks[i]
                    if (sel_t.dtype == FP8 and i + 1 < len(chunks)
                            and chunks[i + 1][0] is g
                            and chunks[i + 1][1] == c + 1):
                        groups.append((g, c, i, True))
                        i += 2
                    else:
                        groups.append((g, c, i, False))
                        i += 1
                for gi, (g, c, i, dbl) in enumerate(groups):
                    st = first and gi == 0
                    sp_ = last and gi == len(groups) - 1
                    if dbl:
                        nc.tensor.matmul(
                            out=ps,
                            lhsT=sel_t[:, (c0 + i) * P:(c0 + i + 2) * P]
                                .rearrange("p (k m) -> p k m", k=2),
                            rhs=g[:, c * D:(c + 2) * D]
                                .rearrange("p (k d) -> p k d", k=2),
                            start=st, stop=sp_, perf_mode=DR,
                        )
                    else:
                        nc.tensor.matmul(
                            out=ps,
                            lhsT=sel_t[:, (c0 + i) * P:(c0 + i + 1) * P],
                            rhs=g[:, c * D:(c + 1) * D],
                            start=st, stop=sp_,
                        )

            def transpose_into(pt, cbase, src_sb, nch):
                for c in range(nch):
                    nc.tensor.transpose(
                        out=pt[:, (cbase + c) * P:(cbase + c + 1) * P],
                        in_=src_sb[:, c * P:(c + 1) * P],
                        identity=ident[:],
                    )

            def linear(xT, kc, w_t, b_row):
                ps = pop.tile([P, D], F32, name="ps_o", tag="ps_o")
                for kk in range(kc):
                    nc.tensor.matmul(
                        out=ps[:], lhsT=xT[:, kk * P:(kk + 1) * P],
                        rhs=w_t[:, kk * D:(kk + 1) * D],
                        start=(kk == 0),
                        stop=(plan.no_bias and kk == kc - 1),
                    )
                if not plan.no_bias:
                    nc.tensor.matmul(out=ps[:], lhsT=ones1[:], rhs=b_row[:],
                                     start=False, stop=True)
                return ps

            GA_BUFS = 2 * (pf + 1)

            # Zero every gather-ring buffer once: skipped -1 tails leave
            # stale SBUF that the selector matmuls read (zero-selector), and
            # uninitialized bits could decode as NaN (0 * NaN = NaN).
            for i in range(GA_BUFS):
                t = gp.tile([P, min(KA_MAX, CMAX) * D], BF16, name="z_ga",
                            tag="ga", bufs=GA_BUFS)
                eng = nc.vector if i % 2 == 0 else nc.gpsimd
                eng.memset(t[:], 0.0)
            for i in range(4):
                t = gp.tile([P, min(KE_MAX, CMAX) * D], FP8, name="z_ge",
                            tag="ge", bufs=4)
                eng = nc.gpsimd if i % 2 == 0 else nc.vector
                eng.memset(t[:], 0.0)

            def node_adj_gather(k, d, b):
                tab = n_tabs[k][d]
                dt = FP8 if k == 0 else BF16
                return gather(tab[:], ixn_t, int(plan.n_off[d, b]),
                              int(Ka[d, b]), "ga", KA_MAX, dt,
                              plan.cnt_a[d, b], bufs=GA_BUFS)

            def node_ge_lo(k, d, b):
                ke0 = int(Ke[0, d, b])
                co = int(plan.n_off[d, b]) + int(Ka[d, b])
                lo = gather(e_tabs[k][0][:], ixn_t, co, ke0, "ge", KE_MAX,
                            FP8, plan.cnt_e[0, d, b], bufs=4)
                return True, lo

            def node_ge_hi(k, d, b, gt):
                ke0, ke1 = int(Ke[0, d, b]), int(Ke[1, d, b])
                co = int(plan.n_off[d, b]) + int(Ka[d, b])
                return gather(e_tabs[k][1][:], ixn_t, co + ke0, ke1, "ge",
                              KE_MAX, FP8, plan.cnt_e[1, d, b], bufs=4)

            def node_block(k, d, b, ga, ge):
                last = (k == K - 1)
                ka = int(Ka[d, b])
                ke0, ke1 = int(Ke[0, d, b]), int(Ke[1, d, b])
                ke = ke0 + ke1
                co = int(plan.n_off[d, b])
                adt = FP8 if k == 0 else BF16
                sel_a = build_sel(dn_t, co, ka, adt, "sela", SELA_MAX)
                sel_e = build_sel(dn_t, co + ka, ke, FP8, "sele", SELE_MAX)
                own = sp.tile([P, D], BF16, name="own", tag="own")
                st = fw_st[k] if d == 0 else bw_st[k]
                nc.sync.dma_start(out=own[:], in_=st[b * P:(b + 1) * P, :])

                ps_m = pmp.tile([P, D], F32, name="ps_m", tag="ps_f")
                sel_matmul(ps_m[:], sel_a, 0, flat_chunks(ga),
                           True, ke == 0)
                sel_matmul(ps_m[:], sel_e, 0, flat_chunks(ge),
                           ka == 0, True)
                m_sb = sp.tile([P, D], BF16, name="m_sb", tag="m")
                nc.scalar.activation(out=m_sb[:], in_=ps_m[:], func=COPY,
                                     scale=rcn_t[:, d * NB + b:d * NB + b + 1])

                pt = ptp.tile([P, KCN * P], BF16, name="pt", tag="pt")
                transpose_into(pt, 0, own[:], DC)
                transpose_into(pt, DC, m_sb[:], DC)
                xT = xp.tile([P, KCN * P], BF16, name="xT", tag="xT")
                nc.vector.tensor_copy(out=xT[:], in_=pt[:])

                w_t = wfc_t if d == 0 else wbc_t
                b_row = bfc_t if d == 0 else bbc_t
                ps_o = linear(xT, KCN, w_t, b_row)
                if not last:
                    ob = op.tile([P, D], BF16, name="ob", tag="ob")
                    nc.scalar.activation(out=ob[:], in_=ps_o[:], func=RELU)
                    obq = op.tile([P, D], FP8, name="obq", tag="obq")
                    nc.scalar.activation(out=obq[:], in_=ps_o[:], func=RELU)
                    dst = fw_st[k + 1] if d == 0 else bw_st[k + 1]
                    dstq = fw_stq[k + 1] if d == 0 else bw_stq[k + 1]
                    nc.sync.dma_start(out=dst[b * P:(b + 1) * P, :], in_=ob[:])
                    nc.sync.dma_start(out=dstq[b * P:(b + 1) * P, :],
                                      in_=obq[:])
                else:
                    of = op.tile([P, D], F32, name="of", tag="of")
                    nc.scalar.activation(out=of[:], in_=ps_o[:], func=COPY)
                    dst = fw_out if d == 0 else bw_out
                    nc.sync.dma_start(out=dst[b * P:(b + 1) * P, :], in_=of[:])

            def edge_gb_gather(k, b):
                kf, kb = int(Kf[b]), int(Kb[b])
                co = int(plan.e_off[b])
                return gather(bw_tabq[k + 1][:], ixe_t, co + kf, kb, "gd",
                              KD_MAX, FP8, plan.cnt_b[b], bufs=8)

            def edge_gf_gather(k, b):
                kf = int(Kf[b])
                co = int(plan.e_off[b])
                return gather(fw_tabq[k + 1][:], ixe_t, co, kf, "gd",
                              KD_MAX, FP8, plan.cnt_f[b], bufs=8)

            def edge_block(k, b, gb, gf):
                kf, kb = int(Kf[b]), int(Kb[b])
                co = int(plan.e_off[b])
                sel_t = build_sel(de_t, co, kf + kb, FP8, "seld", SELD_MAX)
                own = sp.tile([P, D], BF16, name="own_e", tag="own")
                nc.sync.dma_start(out=own[:],
                                  in_=e_st[k][b * P:(b + 1) * P, :])

                # bw half first, fw half second (independent PSUM tiles so
                # each half retires on its own).
                ps_b = pmp.tile([P, D], F32, name="ps_be", tag="ps_b")
                sel_matmul(ps_b[:], sel_t, kf, flat_chunks(gb),
                           True, True)
                mb = sp.tile([P, D], BF16, name="mb", tag="m2")
                nc.scalar.activation(out=mb[:], in_=ps_b[:], func=COPY,
                                     scale=rceb_t[:, b:b + 1])

                ps_f = pmp.tile([P, D], F32, name="ps_fe", tag="ps_f")
                sel_matmul(ps_f[:], sel_t, 0, flat_chunks(gf),
                           True, True)
                mf = sp.tile([P, D], BF16, name="mf", tag="m")
                nc.scalar.activation(out=mf[:], in_=ps_f[:], func=COPY,
                                     scale=rcef_t[:, b:b + 1])

                pt = ptp.tile([P, KCE * P], BF16, name="pt_e", tag="pt")
                transpose_into(pt, 0, own[:], DC)
                transpose_into(pt, DC, mf[:], DC)
                transpose_into(pt, 2 * DC, mb[:], DC)
                xT = xp.tile([P, KCE * P], BF16, name="xT_e", tag="xT")
                nc.vector.tensor_copy(out=xT[:], in_=pt[:])

                ps_o = linear(xT, KCE, wed_t, bed_t)
                eb = op.tile([P, D], BF16, name="eb", tag="ob")
                nc.scalar.activation(out=eb[:], in_=ps_o[:], func=RELU)
                ebq = op.tile([P, D], FP8, name="ebq", tag="obq")
                nc.scalar.activation(out=ebq[:], in_=ps_o[:], func=RELU)
                nc.sync.dma_start(out=e_st[k + 1][b * P:(b + 1) * P, :],
                                  in_=eb[:])
                nc.sync.dma_start(out=e_stq[k + 1][b * P:(b + 1) * P, :],
                                  in_=ebq[:])

            # ---------------- program ----------------
            def prefetch_unit(k, d, b, with_lo):
                ent = {"ga": node_adj_gather(k, d, b), "gt": None, "lo": None}
                if with_lo:
                    ent["gt"], ent["lo"] = node_ge_lo(k, d, b)
                return ent

            units = [(d, b) for d in (1, 0) for b in range(NB)]
            pend = [prefetch_unit(0, *units[j], with_lo=(j < 2))
                    for j in range(pf)]
            EPF = 6
            FPF = 3
            for k in range(K):
                epend = None
                for ui, (d, b) in enumerate(units):
                    if k == 0 and ui == 0:
                        # edge-gather ring is first touched in the edge
                        # phase: zero it during the node phase
                        for i in range(8):
                            t = gp.tile([P, min(KD_MAX, CMAX) * D], FP8,
                                        name="z_gd", tag="gd", bufs=8)
                            nc.vector.memset(t[:], 0.0)
                    ent = pend[ui]
                    if ui + pf < len(units):
                        pend.append(
                            prefetch_unit(k, *units[ui + pf], with_lo=False))
                    if ent["gt"] is None:
                        ent["gt"], ent["lo"] = node_ge_lo(k, d, b)
                    ge = ent["lo"] + node_ge_hi(k, d, b, ent["gt"])
                    node_block(k, d, b, ent["ga"], ge)
                    if k < K - 1 and b == NB - 1:
                        # per-direction fp8 AllGather right behind its last
                        # producing block (bw first, so its AG hides under
                        # the fw node phase)
                        stq = fw_stq[k + 1] if d == 0 else bw_stq[k + 1]
                        tabq = fw_tabq[k + 1] if d == 0 else bw_tabq[k + 1]
                        allgather(stq[:], tabq[:])
                    if k < K - 1 and ui == len(units) - 3:
                        # edge-phase bw-dep gathers depend only on the bwq
                        # AllGather (done mid-fw-phase): issue them before
                        # the last fw blocks so their data is resident when
                        # the edge phase starts.
                        epend = [edge_gb_gather(k, j) for j in range(EPF)]
                if k < K - 1:
                    pend = []
                    fpend = [edge_gf_gather(k, b) for b in range(FPF)]
                    for b in range(EB):
                        if b + EPF < EB:
                            epend.append(edge_gb_gather(k, b + EPF))
                        if b + FPF < EB:
                            fpend.append(edge_gf_gather(k, b + FPF))
                        edge_block(k, b, epend[b], fpend[b])
                        # bf16 node tables are only needed by hop k+1's
                        # adjacency gathers: all-gather them during the edge
                        # phase, behind the critical fp8 AllGathers.
                        if b == 0:
                            allgather(bw_st[k + 1][:], bw_tabs[k + 1][:])
                        elif b == 1:
                            allgather(fw_st[k + 1][:], fw_tabs[k + 1][:])
                        elif b == ES_LO // P - 1:
                            # lo part (5/8) of the edge table: AllGather
                            # overlaps the remaining edge blocks; the exposed
                            # hi AllGather at the hop boundary shrinks.
                            dst = e_tabs[k + 1][0][:].rearrange(
                                "(c r) d -> c r d", r=ES_LO)
                            allgather(e_stq[k + 1][0:ES_LO, :], dst)
                    pend = [prefetch_unit(k + 1, *units[j], with_lo=(j < 2))
                            for j in range(pf)]
                    dst = e_tabs[k + 1][1][:].rearrange("(c r) d -> c r d",
                                                        r=ES_HI)
                    allgather(e_stq[k + 1][ES_LO:ES, :], dst)

    # Rebind SWDGE queue_num to the scheduled DMASW lane so each completion
    # semaphore always fires from one queue (the tile scheduler reorders
    # Pool DMA instructions).
    from concourse.tile_sem_assignment import PROC_NAME_TO_IDX
    idx_to_proc = {v: k for k, v in PROC_NAME_TO_IDX.items()}
    for blk in nc.m.functions[0].blocks:
        for inst in blk.instructions:
            if (inst.engine == mybir.EngineType.Pool
                    and hasattr(inst, "queue_num")
                    and getattr(inst, "bass_scheduled_proc", None) is not None):
                pname = idx_to_proc.get(inst.bass_scheduled_proc, "")
                if isinstance(pname, str) and pname.startswith("DMASW"):
                    inst.queue_num = int(pname[5:]) % 4

    nc.compile()
    return nc


def _pack_idx(lst):
    """[m] int (m % 128 == 0) -> [128, m/16] int16 wrapped gather layout."""
    wrapped = lst.astype(np.int16).reshape(-1, 16).T
    return np.tile(wrapped, (8, 1))


def prep_inputs(cfg: Cfg, plan: Plan, inputs: dict):
    import ml_dtypes
    bf16 = ml_dtypes.bfloat16
    fp8 = ml_dtypes.float8_e4m3
    N, E, D = cfg.N, cfg.E, cfg.D
    NS, ES, NB, EB, C = cfg.NS, cfg.ES, cfg.NB, cfg.EB, cfg.CORES
    KCN, KCE = cfg.KCN, cfg.KCE
    f32 = np.float32

    fw = np.asarray(inputs["fw_input"], f32)
    bw = np.asarray(inputs["bw_input"], f32)
    ee = np.asarray(inputs["edge_embs"], f32)
    adj = {0: np.asarray(inputs["fw_adj"], np.int64),
           1: np.asarray(inputs["bw_adj"], np.int64)}
    eid = {0: np.asarray(inputs["fw_edgeid"], np.int64),
           1: np.asarray(inputs["bw_edgeid"], np.int64)}
    dep = {0: np.asarray(inputs["fw_edgedep"], np.int64),
           1: np.asarray(inputs["bw_edgedep"], np.int64)}

    def wchunks(W, kc):
        W = np.asarray(W, f32)
        return np.concatenate([W[kk * P:(kk + 1) * P, :] for kk in range(kc)],
                              axis=1).astype(bf16)

    wfc = wchunks(inputs["Wfc"], KCN)
    wbc = wchunks(inputs["Wbc"], KCN)
    wed = wchunks(inputs["Wedge"], KCE)
    bfc = np.asarray(inputs["bfc"], f32).reshape(1, D).astype(bf16)
    bbc = np.asarray(inputs["bbc"], f32).reshape(1, D).astype(bf16)
    bed = np.asarray(inputs["bedge"], f32).reshape(1, D).astype(bf16)

    fw_tab0 = fw.astype(fp8)
    bw_tab0 = bw.astype(fp8)
    ES_LO = (ES * 5) // 8
    ES_HI = ES - ES_LO
    ee_r = ee.reshape(C, ES, D)
    e_tab0_lo = ee_r[:, :ES_LO].reshape(C * ES_LO, D).astype(fp8)
    e_tab0_hi = ee_r[:, ES_LO:].reshape(C * ES_HI, D).astype(fp8)

    def e_remap(v):
        # global edge id -> (half, row within half-table)
        cown = v // ES
        j = v % ES
        h = (j >= ES_LO).astype(np.int64)
        return h, np.where(h == 0, cown * ES_LO + j,
                           cown * ES_HI + (j - ES_LO))

    def pad_lists(vals, msk, kch, cnts):
        """valid list -> [kch*128]: valid entries, fake idx-0 fill up to
        each piece's shared count, -1 skip-tail beyond."""
        lst = vals[msk]
        m = kch * P
        lpad = np.full(m, -1, np.int64)
        lpad[:len(lst)] = lst
        off = 0
        for nch, cnt in zip(_pieces(kch), cnts):
            have = min(max(len(lst) - off * P, 0), nch * P)
            lpad[off * P + have: off * P + cnt] = 0
            off += nch
        return lpad

    in_maps = []
    for c in range(C):
        idx_cols = []
        dest_n = np.full((P, plan.n_chunks), -1.0, f32)
        rcn = np.zeros((P, 2 * NB), f32)
        for d in (0, 1):
            for b in range(NB):
                r0 = c * NS + b * P
                ka = int(plan.Ka[d, b])
                co = int(plan.n_off[d, b])
                av = adj[d][r0:r0 + P]
                ev = eid[d][r0:r0 + P]
                am, em = av >= 0, ev >= 0
                rcn[:, d * NB + b] = 1.0 / (am.sum(1) + em.sum(1))
                eh, erow = e_remap(np.maximum(ev, 0))
                eh = np.where(em, eh, -1)
                ke0 = int(plan.Ke[0, d, b])
                for (vals, msk, kch, base, cnts) in (
                        (av, am, ka, co, plan.cnt_a[d, b]),
                        (erow, eh == 0, ke0, co + ka, plan.cnt_e[0, d, b]),
                        (erow, eh == 1, int(plan.Ke[1, d, b]), co + ka + ke0,
                         plan.cnt_e[1, d, b])):
                    pidx, _ = np.nonzero(msk)
                    lst = vals[msk]
                    lpad = pad_lists(vals, msk, kch, cnts)
                    idx_cols.append(_pack_idx(lpad))
                    i = np.arange(len(lst))
                    dest_n[i % P, base + i // P] = pidx
        idx_n = np.concatenate(idx_cols, axis=1)

        idx_cols = []
        dest_e = np.full((P, plan.e_chunks), -1.0, f32)
        rcef = np.zeros((P, EB), f32)
        rceb = np.zeros((P, EB), f32)
        for b in range(EB):
            r0 = c * ES + b * P
            kf, kb = int(plan.Kf[b]), int(plan.Kb[b])
            co = int(plan.e_off[b])
            fv, bv = dep[0][r0:r0 + P], dep[1][r0:r0 + P]
            fm, bm = fv >= 0, bv >= 0
            rcef[:, b] = 1.0 / fm.sum(1)
            rceb[:, b] = 1.0 / bm.sum(1)
            for (vals, msk, kch, base, cnts) in (
                    (fv, fm, kf, 0, plan.cnt_f[b]),
                    (bv, bm, kb, kf, plan.cnt_b[b])):
                pidx, _ = np.nonzero(msk)
                lst = vals[msk]
                lpad = pad_lists(vals, msk, kch, cnts)
                idx_cols.append(_pack_idx(lpad))
                i = np.arange(len(lst))
                dest_e[i % P, co + base + i // P] = pidx
        idx_e = np.concatenate(idx_cols, axis=1)

        im = {
            "fw_tab0": fw_tab0, "bw_tab0": bw_tab0,
            "e_tab0_lo": e_tab0_lo, "e_tab0_hi": e_tab0_hi,
            "fw_own0": fw[c * NS:(c + 1) * NS].astype(bf16),
            "bw_own0": bw[c * NS:(c + 1) * NS].astype(bf16),
            "e_own0": ee[c * ES:(c + 1) * ES].astype(bf16),
            "idx_n": idx_n, "idx_e": idx_e,
            "dest_n": dest_n.astype(bf16), "dest_e": dest_e.astype(bf16),
            "rcn": rcn, "rcef": rcef, "rceb": rceb,
            "wfc": wfc, "wbc": wbc, "wed": wed,
            "bfc": bfc, "bbc": bbc, "bed": bed,
        }
        in_maps.append(im)
    return in_maps


def assemble_outputs(cfg: Cfg, results):
    fw = np.concatenate([results[c]["fw_out"] for c in range(cfg.CORES)], axis=0)
    bw = np.concatenate([results[c]["bw_out"] for c in range(cfg.CORES)], axis=0)
    return fw, bw


# ======================= self-contained runner =======================
import os as _os
import types as _types


def _install_axon_prof():
    name = "antenv.axon_hooks"
    if name in sys.modules:
        return True
    try:
        mod = _types.ModuleType(name)
        mod._hook = None
        mod.set_axon_ntff_profile_hook = lambda h: setattr(mod, "_hook", h)
        mod.get_axon_ntff_profile_hook = lambda: mod._hook
        sys.modules[name] = mod
        import antenv
        antenv.axon_hooks = mod
        from trn_agent_boot.trn_boot import _ntff_profile_via_ctypes
        mod.set_axon_ntff_profile_hook(
            _ntff_profile_via_ctypes('/opt/axon/libaxon_pjrt.so'))
        return True
    except Exception:
        sys.modules.pop(name, None)
        return False


_CACHE = {}
LAST_EXEC_NS = None
LAST_PROFILE = None


def kernel(**inputs):
    """Full-input GNN forward on 8 TRN2 NeuronCores. Returns (fw, bw)."""
    global LAST_EXEC_NS, LAST_PROFILE
    from concourse.bass_utils import run_bass_kernel_spmd

    cfg = Cfg()
    plan = Plan(cfg, inputs)
    key = plan.sig
    if _CACHE.get("key") != key:
        _CACHE["nc"] = build(cfg, plan)
        _CACHE["key"] = key
    nc = _CACHE["nc"]

    in_maps = prep_inputs(cfg, plan, inputs)

    profile = _os.environ.get("GNN_PROFILE", "0") == "1"
    if profile:
        profile = _install_axon_prof()
    res = run_bass_kernel_spmd(nc, in_maps, core_ids=list(range(cfg.CORES)),
                               trace=profile)
    LAST_EXEC_NS = res.exec_time_ns
    LAST_PROFILE = res.profile_json
    if res.instructions_and_trace is not None:
        try:
            print("trace:", res.instructions_and_trace[1])
        except Exception:
            pass
    return assemble_outputs(cfg, res.results)
